# revision 11
# baseline (speedup 1.0000x reference)
"""Dense transformer block (attention + post-LN MLP) on 8 trn2 NeuronCores.

Head-parallel sharding: core c handles heads {2c, 2c+1} of BOTH batches
over the full 2048-token sequence, so every core runs the same uniform
causal-attention program (no runtime Switch) and there is no K/V
exchange: each core computes q/k/v for its own heads over all tokens.
The only collective is ONE 8-rank AllToAll that redistributes attention
outputs so core c = 4b+j receives all 16 heads' outputs for batch b,
tokens [512j, 512j+512) — its MLP shard — at a core-independent
address, in natural head order. MLP is token-sharded. Weights
replicated (bf16); activations feature-major (x^T) end-to-end. qkv /
attention-score / AV / softmax-exp work is interleaved in emission
order so the scalar-engine exp stream hides under tensor-engine
matmuls.
"""

import numpy as np
import ml_dtypes

BF16 = ml_dtypes.bfloat16

N_CORES = 8
B, S, D = 2, 2048, 1024
H, HD = 16, 64
F = 4 * D
TOK = 512            # tokens owned per core (MLP/output shard)
P = 128
NTB = 4              # 512-token blocks per sequence
VW = 66              # per-head V row width: 64 v + ones col + pad
MASK_NEG = -80000.0  # -> -79872 in bf16; /8 => exp underflows to exactly 0
LN_EPS = 1e-5

_CACHE = {}


def _patched_insert_act_table_loads(self):
    """Instance-level replacement for Bacc.insert_act_table_loads that
    removes Exp/Ln from every table set except natural_log_exp_and_others,
    so the softmax reciprocal's Ln/Exp alternation resolves to ONE set and
    the per-head ACT_TABLE_LOAD thrash (~1.3us each) disappears."""
    import bass_rust as _bass_rust
    import concourse.mybir as mybir
    from concourse.hw_specs import get_activation_tables

    has_activation = any(
        isinstance(i, mybir.InstActivation)
        for b in self.main_func.blocks
        for i in b.instructions
    )
    if not has_activation:
        return
    tabs = get_activation_tables(self.m.arch)
    AF = mybir.ActivationFunctionType
    if "natural_log_exp_and_others" in tabs:
        for name, fns in tabs.items():
            if name != "natural_log_exp_and_others":
                fns.discard(AF.Exp)
                fns.discard(AF.Ln)
    _bass_rust.insert_act_table_loads(self, list(tabs.items()))


def _build():
    import types

    import concourse.bass as bass
    import concourse.mybir as mybir
    import concourse.tile as tile
    from concourse import bacc

    dt = mybir.dt
    AF = mybir.ActivationFunctionType
    OP = mybir.AluOpType

    nc = bacc.Bacc(
        "TRN2",
        target_bir_lowering=False,
        debug=False,
        enable_asserts=True,
        num_devices=N_CORES,
    )
    try:
        nc.insert_act_table_loads = types.MethodType(
            _patched_insert_act_table_loads, nc
        )
    except Exception:
        pass

    def din(name, shape, dty):
        return nc.dram_tensor(name, shape, dty, kind="ExternalInput").ap()

    xTb0 = din("xTb0", [D, S], dt.bfloat16)     # batch 0, feature-major
    xTb1 = din("xTb1", [D, S], dt.bfloat16)     # batch 1
    xTf = din("xTf", [D, TOK], dt.float32)      # own token slice (residual)
    w_q = din("w_q", [D, P], dt.bfloat16)       # own 2 heads
    w_k = din("w_k", [D, P], dt.bfloat16)
    w_v = din("w_v", [D, P], dt.bfloat16)
    w_o = din("w_o", [D, D], dt.bfloat16)       # full, replicated
    w_fc = din("w_fc", [D, F], dt.bfloat16)
    w_pr = din("w_pr", [F, D], dt.bfloat16)
    b_q = din("b_q", [P], dt.float32)
    b_k = din("b_k", [P], dt.float32)
    b_o = din("b_o", [D], dt.float32)           # b_o_eff (v-bias folded)
    b_fc = din("b_fc", [F], dt.float32)
    b_pr = din("b_pr", [D], dt.float32)
    g1 = din("g1", [D], dt.float32)
    b1 = din("b1", [D], dt.float32)
    g2 = din("g2", [D], dt.float32)
    b2 = din("b2", [D], dt.float32)
    ident_d = din("ident", [P, P], dt.bfloat16)
    maskm_d = din("maskm", [P, P], dt.bfloat16)
    out_d = nc.dram_tensor("out", [D, TOK], dt.float32, kind="ExternalOutput").ap()

    GROUPS = [[0, 1, 2, 3, 4, 5, 6, 7]]

    with tile.TileContext(nc) as tc:
        from contextlib import ExitStack

        ctx = ExitStack()
        with ctx:
            c_pool = ctx.enter_context(tc.tile_pool(name="consts", bufs=1))
            dram = ctx.enter_context(tc.tile_pool(name="dram", bufs=1, space="DRAM"))

            # ---- constants ----
            ident = c_pool.tile([P, P], dt.bfloat16, tag="ident")
            nc.sync.dma_start(ident[:], ident_d[:])
            maskm = c_pool.tile([P, P], dt.bfloat16, tag="maskm")
            nc.sync.dma_start(maskm[:], maskm_d[:])
            ones128_bf = c_pool.tile([P, 1], dt.bfloat16, tag="ones128")
            nc.vector.memset(ones128_bf[:], 1.0)
            ones1_f = c_pool.tile([1, P], dt.float32, tag="ones1f")
            nc.vector.memset(ones1_f[:], 1.0)
            ones_hi = c_pool.tile([65, 64], dt.bfloat16, tag="oneshi")
            nc.vector.memset(ones_hi[64:65, :], 1.0)

            bq_sb = c_pool.tile([P, 1], dt.float32, tag="bq")
            nc.sync.dma_start(bq_sb[:], b_q.rearrange("(c p) -> p c", p=P))
            bk_sb = c_pool.tile([P, 1], dt.float32, tag="bk")
            nc.sync.dma_start(bk_sb[:], b_k.rearrange("(c p) -> p c", p=P))
            bo_sb = c_pool.tile([P, 8], dt.float32, tag="bo")
            nc.sync.dma_start(bo_sb[:], b_o.rearrange("(c p) -> p c", p=P))
            bfc_sb = c_pool.tile([P, 32], dt.float32, tag="bfc")
            nc.sync.dma_start(bfc_sb[:], b_fc.rearrange("(c p) -> p c", p=P))
            bpr_sb = c_pool.tile([P, 8], dt.float32, tag="bpr")
            nc.sync.dma_start(bpr_sb[:], b_pr.rearrange("(c p) -> p c", p=P))
            g1_sb = c_pool.tile([P, 8], dt.float32, tag="g1")
            nc.sync.dma_start(g1_sb[:], g1.rearrange("(c p) -> p c", p=P))
            b1_sb = c_pool.tile([P, 8], dt.float32, tag="b1")
            nc.sync.dma_start(b1_sb[:], b1.rearrange("(c p) -> p c", p=P))
            g2_sb = c_pool.tile([P, 8], dt.float32, tag="g2")
            nc.sync.dma_start(g2_sb[:], g2.rearrange("(c p) -> p c", p=P))
            b2_sb = c_pool.tile([P, 8], dt.float32, tag="b2")
            nc.sync.dma_start(b2_sb[:], b2.rearrange("(c p) -> p c", p=P))

            # ---- AllToAll buffers (one 8-rank op) ----
            a2a_in = dram.tile([8 * P * TOK], dt.bfloat16, tag="a2ai", name="a2ai")
            a2a_out = dram.tile([8, P * TOK], dt.bfloat16, tag="a2ao", name="a2ao")

            # ---- long-lived activations ----
            pR_cm = tc.tile_pool(name="pR", bufs=1)
            pR = pR_cm.__enter__()
            r1 = pR.tile([P, 8, TOK], dt.float32, tag="r1")

            pA_cm = tc.tile_pool(name="pA", bufs=1)
            pA = pA_cm.__enter__()
            qT = pA.tile([P, 2, S], dt.bfloat16, tag="qT")
            kT = pA.tile([P, 2, S], dt.bfloat16, tag="kT")
            v_ones = pA.tile([P, 2, 16, 2 * VW], dt.bfloat16, tag="v_ones")
            o_cat = [pA.tile([P, S], dt.bfloat16, tag=f"o_cat{i}", name=f"o_cat{i}")
                     for i in range(2)]
            o_g = pA.tile([P, 8, TOK], dt.bfloat16, tag="o_g")
            wo_sb = pA.tile([P, 8, D], dt.bfloat16, tag="wo")
            xf = pA.tile([P, 8, TOK], dt.float32, tag="xf")

            v5 = v_ones.rearrange("p b k (h w) -> p b k h w", w=VW)
            nc.vector.memset(v5[:, :, :, :, 64:66], 0.0)
            nc.vector.memset(v5[:, :, :, :, 64:65], 1.0)

            # ---- qkv inputs ----
            xw_cm = tc.tile_pool(name="xw", bufs=1)
            xw = xw_cm.__enter__()
            wk_sb = xw.tile([P, 8, P], dt.bfloat16, tag="wk")
            nc.sync.dma_start(wk_sb[:], w_k.rearrange("(c p) f -> p c f", p=P))
            wq_sb = xw.tile([P, 8, P], dt.bfloat16, tag="wq")
            nc.sync.dma_start(wq_sb[:], w_q.rearrange("(c p) f -> p c f", p=P))
            wv_sb = xw.tile([P, 8, P], dt.bfloat16, tag="wv")
            nc.sync.dma_start(wv_sb[:], w_v.rearrange("(c p) f -> p c f", p=P))
            xbp_cm = tc.tile_pool(name="xbp", bufs=3)
            xbp = xbp_cm.__enter__()
            xbv = [xTb0.rearrange("(c p) t -> p c t", p=P),
                   xTb1.rearrange("(c p) t -> p c t", p=P)]

            # prefetches for the post-attention phases
            nc.sync.dma_start(xf[:], xTf.rearrange("(c p) t -> p c t", p=P))
            nc.sync.dma_start(wo_sb[:], w_o.rearrange("(c p) f -> p c f", p=P))

            # ---- attention working pools ----
            # qkv psum shares the scores pool ("sps" tag): 2 slots x 2 banks
            # + oT 2 slots x 2 banks = all 8 banks.
            s_ps_cm = tc.tile_pool(name="att_s", bufs=2, space="PSUM")
            s_ps = s_ps_cm.__enter__()
            o_ps_cm = tc.tile_pool(name="att_o", bufs=2, space="PSUM")
            o_ps = o_ps_cm.__enter__()
            atmp_cm = tc.tile_pool(name="att_tmp", bufs=3)
            atmp = atmp_cm.__enter__()
            atmp2_cm = tc.tile_pool(name="att_tmp2", bufs=2)
            atmp2 = atmp2_cm.__enter__()

            def qkv_tb(bb, tb):
                """q/k/v for batch bb, token block tb (512 tokens), 2 heads."""
                xbt = xbp.tile([P, 8, TOK], dt.bfloat16, tag="xb")
                nc.sync.dma_start(
                    xbt[:], xbv[bb][:, :, tb * TOK:(tb + 1) * TOK])
                ps = s_ps.tile([P, TOK], dt.float32, tag="sps")
                for c8 in range(8):
                    nc.tensor.matmul(
                        ps[:], lhsT=wk_sb[:, c8, :], rhs=xbt[:, c8, :],
                        start=(c8 == 0), stop=(c8 == 7),
                    )
                nc.vector.tensor_scalar(
                    kT[:, bb, tb * TOK:(tb + 1) * TOK], ps[:],
                    bk_sb[:, 0:1], None, OP.add,
                )
                for kb in range(4):
                    kcg = tb * 4 + kb
                    ps = s_ps.tile([P, P], dt.float32, tag="sps")
                    for c8 in range(8):
                        nc.tensor.matmul(
                            ps[:],
                            lhsT=xbt[:, c8, kb * P:(kb + 1) * P],
                            rhs=wv_sb[:, c8, :],
                            start=(c8 == 0), stop=(c8 == 7),
                        )
                    nc.vector.tensor_copy(
                        v5[:, bb, kcg, :, 0:64],
                        ps[:].rearrange("p (h w) -> p h w", w=64),
                    )
                ps = s_ps.tile([P, TOK], dt.float32, tag="sps")
                for c8 in range(8):
                    nc.tensor.matmul(
                        ps[:], lhsT=wq_sb[:, c8, :], rhs=xbt[:, c8, :],
                        start=(c8 == 0), stop=(c8 == 7),
                    )
                nc.vector.tensor_scalar(
                    qT[:, bb, tb * TOK:(tb + 1) * TOK], ps[:],
                    bq_sb[:, 0:1], None, OP.add,
                )

            def attn_block(bb, g):
                """Causal attention for batch bb, query block g (512 q)."""
                nkc = 4 * g + 4
                oT = o_ps.tile([65, 2, TOK], dt.float32, tag="oT")
                for kc in range(nkc):
                    diag = kc >= 4 * g
                    qs = (kc - 4 * g) * P if diag else 0
                    sps = s_ps.tile([P, 2, TOK], dt.float32, tag="sps")
                    for h2 in range(2):
                        nc.tensor.matmul(
                            sps[:, h2, qs:],
                            lhsT=kT[64 * h2:64 * (h2 + 1), bb, kc * P:(kc + 1) * P],
                            rhs=qT[64 * h2:64 * (h2 + 1), bb,
                                   g * TOK + qs:(g + 1) * TOK],
                            start=True, stop=not diag,
                            tile_position=(64 * h2, 0),
                        )
                        if diag:
                            nc.tensor.matmul(
                                sps[:, h2, qs:qs + P],
                                lhsT=ident[:], rhs=maskm[:],
                                start=False, stop=True,
                            )
                    aT = atmp.tile([P, 2, TOK], dt.bfloat16, tag="aT")
                    nc.scalar.activation(
                        aT[:, :, qs:], sps[:, :, qs:], AF.Exp, scale=0.125
                    )
                    for h2 in range(2):
                        nc.tensor.matmul(
                            oT[:, h2, qs:],
                            lhsT=v_ones[:, bb, kc, h2 * VW:h2 * VW + 65],
                            rhs=aT[:, h2, qs:],
                            start=(kc == 0), stop=(kc == nkc - 1),
                        )
                # softmax denominators -> reciprocals via ln/exp, broadcast
                lnrow = atmp2.tile([65, 2, TOK], dt.float32, tag="lnrow")
                nc.scalar.activation(lnrow[64:65, :, :], oT[64:65, :, :], AF.Ln)
                rrow = atmp2.tile([65, 2, TOK], dt.bfloat16, tag="rrow")
                nc.scalar.activation(
                    rrow[64:65, :, :], lnrow[64:65, :, :], AF.Exp, scale=-1.0
                )
                rep = s_ps.tile([P, 2, TOK], dt.float32, tag="sps")
                for h2 in range(2):
                    nc.tensor.matmul(
                        rep[0:64, h2, :],
                        lhsT=ones_hi[64:65, :], rhs=rrow[64:65, h2, :],
                        start=True, stop=True,
                    )
                rep_sb = atmp2.tile([64, 2, TOK], dt.float32, tag="rep_sb")
                nc.scalar.activation(rep_sb[:], rep[0:64, :, :], AF.Copy)
                nc.vector.tensor_tensor(
                    o_cat[bb][0:64, g * TOK:(g + 1) * TOK],
                    oT[0:64, 0, :], rep_sb[:, 0, :], OP.mult,
                )
                ot = atmp2.tile([64, TOK], dt.bfloat16, tag="o_tmp")
                nc.vector.tensor_tensor(ot[:], oT[0:64, 1, :], rep_sb[:, 1, :],
                                        OP.mult)
                nc.sync.dma_start(o_cat[bb][64:128, g * TOK:(g + 1) * TOK], ot[:])

            # ---- interleaved qkv + attention schedule ----
            for g in range(NTB):
                for bb in range(2):
                    qkv_tb(bb, g)
                    attn_block(bb, g)

            # ---- one 8-rank AllToAll: slot p = o for core p's (batch, toks)
            a2av = a2a_in.rearrange("(s p t) -> p s t", s=8, p=P)
            for bb in range(2):
                nc.sync.dma_start(
                    a2av[:, 4 * bb:4 * (bb + 1), :],
                    o_cat[bb].rearrange("p (s t) -> p s t", s=4),
                )
            nc.gpsimd.collective_compute(
                "AllToAll", mybir.AluOpType.bypass,
                replica_groups=GROUPS,
                ins=[a2a_in.opt()], outs=[a2a_out.opt()],
            )

            # close attention pools in LIFO order
            atmp2_cm.__exit__(None, None, None)
            atmp_cm.__exit__(None, None, None)
            o_ps_cm.__exit__(None, None, None)
            s_ps_cm.__exit__(None, None, None)
            xbp_cm.__exit__(None, None, None)
            xw_cm.__exit__(None, None, None)

            # gathered attention outputs: chunk r = heads {2r,2r+1}
            for c8 in range(8):
                nc.sync.dma_start(
                    o_g[:, c8, :],
                    a2a_out[c8].rearrange("(p t) -> p t", p=P),
                )

            # ============ attention projection + residual ============
            with tc.tile_pool(name="wo_ps", bufs=4, space="PSUM") as wo_ps:
                for oc in range(8):
                    ps = wo_ps.tile([P, TOK], dt.float32, tag="wops")
                    for c8 in range(8):
                        nc.tensor.matmul(
                            ps[:],
                            lhsT=wo_sb[:, c8, oc * P:(oc + 1) * P],
                            rhs=o_g[:, c8, :],
                            start=(c8 == 0), stop=(c8 == 7),
                        )
                    nc.vector.scalar_tensor_tensor(
                        r1[:, oc, :], ps[:], bo_sb[:, oc:oc + 1], xf[:, oc, :],
                        op0=OP.add, op1=OP.add,
                    )

            # ============ layernorm (feature-major, partition reduce) ====
            def layernorm(src, dst, g_sb, b_sb, tag):
                with (
                    tc.tile_pool(name=f"ln_{tag}", bufs=2) as lnp,
                    tc.tile_pool(name=f"lnps_{tag}", bufs=2, space="PSUM") as lnps,
                    tc.tile_pool(name=f"lnrep_{tag}", bufs=1, space="PSUM") as lnrep,
                ):
                    src_bf = lnp.tile([P, 8, TOK], dt.bfloat16, tag=f"srcbf_{tag}")
                    for c8 in range(8):
                        nc.vector.tensor_copy(src_bf[:, c8, :], src[:, c8, :])
                    s1 = lnps.tile([1, TOK], dt.float32, tag=f"s1_{tag}")
                    s2 = lnps.tile([1, TOK], dt.float32, tag=f"s2_{tag}")
                    for c8 in range(8):
                        sq = lnp.tile([P, TOK], dt.bfloat16, tag=f"sq_{tag}")
                        nc.vector.tensor_tensor(sq[:], src[:, c8, :], src[:, c8, :], OP.mult)
                        nc.tensor.matmul(
                            s1[:], lhsT=ones128_bf[:], rhs=src_bf[:, c8, :],
                            start=(c8 == 0), stop=(c8 == 7),
                        )
                        nc.tensor.matmul(
                            s2[:], lhsT=ones128_bf[:], rhs=sq[:],
                            start=(c8 == 0), stop=(c8 == 7),
                        )
                    mu = lnp.tile([1, TOK], dt.float32, tag=f"mu_{tag}")
                    nc.vector.tensor_scalar(mu[:], s1[:], 1.0 / D, None, OP.mult)
                    m2 = lnp.tile([1, TOK], dt.float32, tag=f"m2_{tag}")
                    nc.vector.tensor_scalar(m2[:], s2[:], 1.0 / D, LN_EPS, OP.mult, OP.add)
                    var = lnp.tile([1, TOK], dt.float32, tag=f"var_{tag}")
                    nc.vector.tensor_tensor(var[:], mu[:], mu[:], OP.mult)
                    nc.vector.tensor_tensor(var[:], m2[:], var[:], OP.subtract)
                    lnv = lnp.tile([1, TOK], dt.float32, tag=f"lnv_{tag}")
                    nc.scalar.activation(lnv[:], var[:], AF.Ln)
                    rsmu = lnp.tile([1, 2, TOK], dt.float32, tag=f"rsmu_{tag}")
                    nc.scalar.activation(rsmu[:, 0, :], lnv[:], AF.Exp, scale=-0.5)
                    nc.vector.tensor_tensor(
                        rsmu[:, 1, :], mu[:], rsmu[:, 0, :], OP.mult
                    )
                    rep = lnrep.tile([P, 2, TOK], dt.float32, tag=f"rep_{tag}")
                    for i in range(2):
                        nc.tensor.matmul(
                            rep[:, i, :], lhsT=ones1_f[:], rhs=rsmu[:, i, :],
                            start=True, stop=True,
                        )
                    rep_sb = lnp.tile([P, 2, TOK], dt.float32, tag=f"repsb_{tag}")
                    nc.scalar.activation(rep_sb[:], rep[:], AF.Copy)
                    for c8 in range(8):
                        t = lnp.tile([P, TOK], dt.float32, tag=f"t_{tag}")
                        nc.vector.tensor_tensor(
                            t[:], src[:, c8, :], rep_sb[:, 0, :], OP.mult
                        )
                        nc.vector.tensor_tensor(t[:], t[:], rep_sb[:, 1, :], OP.subtract)
                        nc.vector.tensor_scalar(
                            dst[:, c8, :], t[:], g_sb[:, c8:c8 + 1], b_sb[:, c8:c8 + 1],
                            OP.mult, OP.add,
                        )

            pA_cm.__exit__(None, None, None)
            pN_cm = tc.tile_pool(name="pN", bufs=1)
            pN = pN_cm.__enter__()
            n_sb = pN.tile([P, 8, TOK], dt.float32, tag="n_sb")
            n_bf = [pN.tile([P, TOK], dt.bfloat16, tag=f"n_bf{i}", name=f"n_bf{i}")
                    for i in range(8)]
            h1g = pN.tile([P, 32, TOK], dt.bfloat16, tag="h1g")
            r2 = pN.tile([P, 8, TOK], dt.float32, tag="r2")

            # MLP weight pools + prefetches (DMAs overlap w_o/LN1)
            wfcp_cm = tc.tile_pool(name="wfc", bufs=2)
            wfcp = wfcp_cm.__enter__()

            def load_wfc(oq):
                wq_ = wfcp.tile([P, 8, 1024], dt.bfloat16, tag="wfcq")
                nc.sync.dma_start(
                    wq_[:],
                    w_fc[:, oq * 1024:(oq + 1) * 1024].rearrange(
                        "(c p) f -> p c f", p=P),
                )
                return wq_

            wprp_cm = tc.tile_pool(name="wpr", bufs=2)
            wprp = wprp_cm.__enter__()

            def load_wpr(q4):
                wq_ = wprp.tile([P, 8, D], dt.bfloat16, tag="wprq")
                nc.sync.dma_start(
                    wq_[:],
                    w_pr[q4 * 1024:(q4 + 1) * 1024, :].rearrange(
                        "(c p) f -> p c f", p=P),
                )
                return wq_

            wfc_q0 = load_wfc(0)
            wpr_q0 = load_wpr(0)  # prefetch during fc

            layernorm(r1, n_sb, g1_sb, b1_sb, "ln1")
            for c8 in range(8):
                nc.vector.tensor_copy(n_bf[c8][:], n_sb[:, c8, :])

            with tc.tile_pool(name="fc_ps", bufs=4, space="PSUM") as fc_ps:
                for oq in range(4):
                    wq_ = wfc_q0 if oq == 0 else load_wfc(oq)
                    for oc8 in range(8):
                        oc = oq * 8 + oc8
                        ps = fc_ps.tile([P, TOK], dt.float32, tag="fcps")
                        for c8 in range(8):
                            nc.tensor.matmul(
                                ps[:],
                                lhsT=wq_[:, c8, oc8 * P:(oc8 + 1) * P],
                                rhs=n_bf[c8][:],
                                start=(c8 == 0), stop=(c8 == 7),
                            )
                        nc.scalar.activation(
                            h1g[:, oc, :], ps[:], AF.Gelu,
                            bias=bfc_sb[:, oc:oc + 1],
                        )

            with tc.tile_pool(name="pr_ps", bufs=1, space="PSUM") as pr_ps:
                mps = [pr_ps.tile([P, TOK], dt.float32, tag=f"mps{i}", name=f"mps{i}")
                       for i in range(8)]
                for q4 in range(4):
                    wq_ = wpr_q0 if q4 == 0 else load_wpr(q4)
                    for oc in range(8):
                        for c8 in range(8):
                            nc.tensor.matmul(
                                mps[oc][:],
                                lhsT=wq_[:, c8, oc * P:(oc + 1) * P],
                                rhs=h1g[:, q4 * 8 + c8, :],
                                start=(q4 == 0 and c8 == 0),
                                stop=(q4 == 3 and c8 == 7),
                            )
                for oc in range(8):
                    nc.vector.scalar_tensor_tensor(
                        r2[:, oc, :], mps[oc][:], bpr_sb[:, oc:oc + 1], n_sb[:, oc, :],
                        op0=OP.add, op1=OP.add,
                    )
            wprp_cm.__exit__(None, None, None)
            wfcp_cm.__exit__(None, None, None)

            layernorm(r2, r2, g2_sb, b2_sb, "ln2")
            out_v = out_d.rearrange("(c p) t -> p c t", p=P)
            for c8 in range(8):
                nc.sync.dma_start(out_v[:, c8, :], r2[:, c8, :])
            pN_cm.__exit__(None, None, None)
            pR_cm.__exit__(None, None, None)

    nc.compile()
    return nc


def _prep_shared(w_attn, b_attn, w_o, b_o, ln1_g, ln1_b, w_fc, b_fc, w_pr, b_pr,
                 ln2_g, ln2_b):
    w_attn = np.asarray(w_attn, np.float32)
    b_attn = np.asarray(b_attn, np.float32)
    w_o_f = np.asarray(w_o, np.float32)
    b_v = b_attn[2 * D:]
    b_o_eff = (np.asarray(b_o, np.float32) + b_v @ w_o_f).astype(np.float32)
    mask = np.where(
        np.arange(P)[:, None] > np.arange(P)[None, :], MASK_NEG, 0.0
    ).astype(BF16)  # [ki, qj]: mask keys above the diagonal
    shared = {
        "w_o": w_o_f.astype(BF16),
        "w_fc": np.asarray(w_fc, np.float32).astype(BF16),
        "w_pr": np.asarray(w_pr, np.float32).astype(BF16),
        "b_o": b_o_eff,
        "b_fc": np.asarray(b_fc, np.float32),
        "b_pr": np.asarray(b_pr, np.float32),
        "g1": np.asarray(ln1_g, np.float32),
        "b1": np.asarray(ln1_b, np.float32),
        "g2": np.asarray(ln2_g, np.float32),
        "b2": np.asarray(ln2_b, np.float32),
        "ident": np.eye(P, dtype=np.float32).astype(BF16),
        "maskm": mask,
    }
    return shared, w_attn, b_attn


def kernel(x, w_attn, b_attn, w_o, b_o, ln1_g, ln1_b, w_fc, b_fc, w_pr, b_pr,
           ln2_g, ln2_b, _trace=False):
    from concourse.bass_utils import run_bass_kernel_spmd

    if "nc" not in _CACHE:
        _CACHE["nc"] = _build()
    nc = _CACHE["nc"]

    x = np.asarray(x, np.float32)
    shared, w_attn_f, b_attn_f = _prep_shared(
        w_attn, b_attn, w_o, b_o, ln1_g, ln1_b, w_fc, b_fc, w_pr, b_pr,
        ln2_g, ln2_b)

    xTb_bf = [np.ascontiguousarray(x[b].T).astype(BF16) for b in range(B)]

    in_maps = []
    for c in range(N_CORES):
        b, j = c // 4, c % 4
        m = dict(shared)
        m["xTb0"] = xTb_bf[0]
        m["xTb1"] = xTb_bf[1]
        m["xTf"] = np.ascontiguousarray(x[b, TOK * j:TOK * (j + 1), :].T)
        m["w_q"] = np.ascontiguousarray(
            w_attn_f[:, P * c:P * (c + 1)]).astype(BF16)
        m["w_k"] = np.ascontiguousarray(
            w_attn_f[:, D + P * c:D + P * (c + 1)]).astype(BF16)
        m["w_v"] = np.ascontiguousarray(
            w_attn_f[:, 2 * D + P * c:2 * D + P * (c + 1)]).astype(BF16)
        m["b_q"] = np.ascontiguousarray(b_attn_f[P * c:P * (c + 1)])
        m["b_k"] = np.ascontiguousarray(b_attn_f[D + P * c:D + P * (c + 1)])
        in_maps.append(m)

    res = run_bass_kernel_spmd(
        nc, in_maps, core_ids=list(range(N_CORES)), trace=_trace
    )
    if _trace:
        _CACHE["exec_time_ns"] = res.exec_time_ns
        _CACHE["insts_and_trace"] = res.instructions_and_trace

    out = np.empty((B, S, D), np.float32)
    for c in range(N_CORES):
        b, j = c // 4, c % 4
        out[b, TOK * j:TOK * (j + 1), :] = res.results[c]["out"].T
    return out


# revision 16
# speedup vs baseline: 1.1075x; 1.1075x over previous
"""Dense transformer block (attention + post-LN MLP) on 8 trn2 NeuronCores.

Head-parallel sharding: core c = 4b+j handles heads {4j..4j+3} of batch
b over the full 2048-token sequence, so every core runs the same
uniform causal-attention program (no runtime Switch) and there is no
K/V exchange: each core computes q/k/v for its own heads over all
tokens. Each core also computes its heads' partial contribution to the
attention projection (o @ w_o rows) for ALL tokens; four pipelined
ReduceScatters (one per 512-token block, issued as each block's
attention completes) sum the partials across the 4-core batch group and
deliver each core its own 128-token segment — at a core-independent
address. The MLP is token-sharded over the RS-assigned tokens. Weights
replicated (bf16); activations feature-major (x^T) end-to-end. qkv /
attention / softmax-exp / w_o-partial work is interleaved in emission
order so the scalar-engine exp stream hides under tensor-engine
matmuls.
"""

import numpy as np
import ml_dtypes

BF16 = ml_dtypes.bfloat16

N_CORES = 8
B, S, D = 2, 2048, 1024
H, HD = 16, 64
F = 4 * D
TOK = 512            # tokens owned per core (MLP/output shard)
P = 128
NTB = 4              # 512-token blocks per sequence
VW = 66              # per-head V row width: 64 v + ones col + pad
MASK_NEG = -80000.0  # -> -79872 in bf16; /8 => exp underflows to exactly 0
LN_EPS = 1e-5

_CACHE = {}


def _patched_insert_act_table_loads(self):
    """Instance-level replacement for Bacc.insert_act_table_loads that
    removes Exp/Ln from every table set except natural_log_exp_and_others,
    so the softmax reciprocal's Ln/Exp alternation resolves to ONE set and
    the per-head ACT_TABLE_LOAD thrash (~1.3us each) disappears."""
    import bass_rust as _bass_rust
    import concourse.mybir as mybir
    from concourse.hw_specs import get_activation_tables

    has_activation = any(
        isinstance(i, mybir.InstActivation)
        for b in self.main_func.blocks
        for i in b.instructions
    )
    if not has_activation:
        return
    tabs = get_activation_tables(self.m.arch)
    AF = mybir.ActivationFunctionType
    if "natural_log_exp_and_others" in tabs:
        for name, fns in tabs.items():
            if name != "natural_log_exp_and_others":
                fns.discard(AF.Exp)
                fns.discard(AF.Ln)
    _bass_rust.insert_act_table_loads(self, list(tabs.items()))


def _build():
    import types

    import concourse.bass as bass
    import concourse.mybir as mybir
    import concourse.tile as tile
    from concourse import bacc

    dt = mybir.dt
    AF = mybir.ActivationFunctionType
    OP = mybir.AluOpType

    nc = bacc.Bacc(
        "TRN2",
        target_bir_lowering=False,
        debug=False,
        enable_asserts=True,
        num_devices=N_CORES,
    )
    try:
        nc.insert_act_table_loads = types.MethodType(
            _patched_insert_act_table_loads, nc
        )
    except Exception:
        pass

    def din(name, shape, dty):
        return nc.dram_tensor(name, shape, dty, kind="ExternalInput").ap()

    xTb = din("xTb", [D, S], dt.bfloat16)       # own batch, feature-major
    xTf = din("xTf", [D, TOK], dt.float32)      # own (RS-order) tokens
    w_q = din("w_q", [D, 256], dt.bfloat16)     # own 4 heads
    w_k = din("w_k", [D, 256], dt.bfloat16)
    w_v = din("w_v", [D, 256], dt.bfloat16)
    w_oo = din("w_oo", [256, D], dt.bfloat16)   # w_o rows of own heads
    w_fc = din("w_fc", [D, F], dt.bfloat16)
    w_pr = din("w_pr", [F, D], dt.bfloat16)
    b_q = din("b_q", [256], dt.float32)
    b_k = din("b_k", [256], dt.float32)
    b_o = din("b_o", [D], dt.float32)           # b_o_eff (v-bias folded)
    b_fc = din("b_fc", [F], dt.float32)
    b_pr = din("b_pr", [D], dt.float32)
    g1 = din("g1", [D], dt.float32)
    b1 = din("b1", [D], dt.float32)
    g2 = din("g2", [D], dt.float32)
    b2 = din("b2", [D], dt.float32)
    ident_d = din("ident", [P, P], dt.bfloat16)
    maskm_d = din("maskm", [P, P], dt.bfloat16)
    out_d = nc.dram_tensor("out", [D, TOK], dt.float32, kind="ExternalOutput").ap()

    GROUPS = [[0, 1, 2, 3], [4, 5, 6, 7]]

    with tile.TileContext(nc) as tc:
        from contextlib import ExitStack

        ctx = ExitStack()
        with ctx:
            c_pool = ctx.enter_context(tc.tile_pool(name="consts", bufs=1))
            dram = ctx.enter_context(tc.tile_pool(name="dram", bufs=1, space="DRAM"))

            # ---- constants ----
            ident = c_pool.tile([P, P], dt.bfloat16, tag="ident")
            nc.sync.dma_start(ident[:], ident_d[:])
            maskm = c_pool.tile([P, P], dt.bfloat16, tag="maskm")
            nc.sync.dma_start(maskm[:], maskm_d[:])
            ones128_bf = c_pool.tile([P, 1], dt.bfloat16, tag="ones128")
            nc.vector.memset(ones128_bf[:], 1.0)
            ones1_f = c_pool.tile([1, P], dt.float32, tag="ones1f")
            nc.vector.memset(ones1_f[:], 1.0)
            ones_hi = c_pool.tile([65, 64], dt.bfloat16, tag="oneshi")
            nc.vector.memset(ones_hi[64:65, :], 1.0)

            bq_sb = c_pool.tile([P, 2], dt.float32, tag="bq")
            nc.sync.dma_start(bq_sb[:], b_q.rearrange("(c p) -> p c", p=P))
            bk_sb = c_pool.tile([P, 2], dt.float32, tag="bk")
            nc.sync.dma_start(bk_sb[:], b_k.rearrange("(c p) -> p c", p=P))
            bo_sb = c_pool.tile([P, 8], dt.float32, tag="bo")
            nc.sync.dma_start(bo_sb[:], b_o.rearrange("(c p) -> p c", p=P))
            bfc_sb = c_pool.tile([P, 32], dt.float32, tag="bfc")
            nc.sync.dma_start(bfc_sb[:], b_fc.rearrange("(c p) -> p c", p=P))
            bpr_sb = c_pool.tile([P, 8], dt.float32, tag="bpr")
            nc.sync.dma_start(bpr_sb[:], b_pr.rearrange("(c p) -> p c", p=P))
            g1_sb = c_pool.tile([P, 8], dt.float32, tag="g1")
            nc.sync.dma_start(g1_sb[:], g1.rearrange("(c p) -> p c", p=P))
            b1_sb = c_pool.tile([P, 8], dt.float32, tag="b1")
            nc.sync.dma_start(b1_sb[:], b1.rearrange("(c p) -> p c", p=P))
            g2_sb = c_pool.tile([P, 8], dt.float32, tag="g2")
            nc.sync.dma_start(g2_sb[:], g2.rearrange("(c p) -> p c", p=P))
            b2_sb = c_pool.tile([P, 8], dt.float32, tag="b2")
            nc.sync.dma_start(b2_sb[:], b2.rearrange("(c p) -> p c", p=P))

            # ---- ReduceScatter buffers: one per 512-token block ----
            rs_in = [dram.tile([4 * D * P], dt.bfloat16, tag=f"rsi{g}",
                               name=f"rsi{g}") for g in range(NTB)]
            rs_out = [dram.tile([D * P], dt.bfloat16, tag=f"rso{g}",
                                name=f"rso{g}") for g in range(NTB)]

            # tiny warm-up collective: absorbs the runtime's first-
            # collective global barrier while qkv is still starting up,
            # so the pipelined RS chain below isn't delayed behind it.
            wu_in = dram.tile([256], dt.bfloat16, tag="wui", name="wui")
            wu_out = dram.tile([4, 256], dt.bfloat16, tag="wuo", name="wuo")
            nc.gpsimd.collective_compute(
                "AllGather", mybir.AluOpType.bypass,
                replica_groups=GROUPS,
                ins=[wu_in.opt()], outs=[wu_out.opt()],
            )

            # ---- long-lived activations ----
            pR_cm = tc.tile_pool(name="pR", bufs=1)
            pR = pR_cm.__enter__()
            r1 = pR.tile([P, 8, TOK], dt.float32, tag="r1")

            pA_cm = tc.tile_pool(name="pA", bufs=1)
            pA = pA_cm.__enter__()
            qT = pA.tile([P, 2, S], dt.bfloat16, tag="qT")
            kT = pA.tile([P, 2, S], dt.bfloat16, tag="kT")
            v_ones = pA.tile([P, 16, 4 * VW], dt.bfloat16, tag="v_ones")
            o_cat = [pA.tile([P, S], dt.bfloat16, tag=f"o_cat{i}", name=f"o_cat{i}")
                     for i in range(2)]
            woo_sb = pA.tile([P, 2, D], dt.bfloat16, tag="woo")
            xf = pA.tile([P, 8, TOK], dt.float32, tag="xf")
            rsg = pA.tile([P, 8, TOK], dt.bfloat16, tag="rsg")

            v4 = v_ones.rearrange("p k (h w) -> p k h w", w=VW)
            nc.vector.memset(v4[:, :, :, 64:66], 0.0)
            nc.vector.memset(v4[:, :, :, 64:65], 1.0)

            # ---- qkv inputs ----
            xw_cm = tc.tile_pool(name="xw", bufs=1)
            xw = xw_cm.__enter__()
            wk_sb = xw.tile([P, 8, 256], dt.bfloat16, tag="wk")
            nc.sync.dma_start(wk_sb[:], w_k.rearrange("(c p) f -> p c f", p=P))
            wq_sb = xw.tile([P, 8, 256], dt.bfloat16, tag="wq")
            nc.sync.dma_start(wq_sb[:], w_q.rearrange("(c p) f -> p c f", p=P))
            wv_sb = xw.tile([P, 8, 256], dt.bfloat16, tag="wv")
            nc.sync.dma_start(wv_sb[:], w_v.rearrange("(c p) f -> p c f", p=P))
            nc.sync.dma_start(woo_sb[:], w_oo.rearrange("(c p) f -> p c f", p=P))
            xbp_cm = tc.tile_pool(name="xbp", bufs=3)
            xbp = xbp_cm.__enter__()
            xbv = xTb.rearrange("(c p) t -> p c t", p=P)

            # prefetch for the post-attention phases
            nc.sync.dma_start(xf[:], xTf.rearrange("(c p) t -> p c t", p=P))

            # ---- attention working pools ----
            # qkv/w_o psum shares the scores pool ("sps" tag): 2 slots x
            # 2 banks + oT 2 slots x 2 banks = all 8 banks.
            s_ps_cm = tc.tile_pool(name="att_s", bufs=2, space="PSUM")
            s_ps = s_ps_cm.__enter__()
            o_ps_cm = tc.tile_pool(name="att_o", bufs=2, space="PSUM")
            o_ps = o_ps_cm.__enter__()
            atmp_cm = tc.tile_pool(name="att_tmp", bufs=3)
            atmp = atmp_cm.__enter__()
            atmp2_cm = tc.tile_pool(name="att_tmp2", bufs=2)
            atmp2 = atmp2_cm.__enter__()
            wop_cm = tc.tile_pool(name="wop", bufs=2)
            wop = wop_cm.__enter__()

            def qkv_tb(tb):
                """q/k/v for token block tb (512 tokens), own 4 heads."""
                xbt = xbp.tile([P, 8, TOK], dt.bfloat16, tag="xb")
                nc.sync.dma_start(
                    xbt[:], xbv[:, :, tb * TOK:(tb + 1) * TOK])
                for hp in range(2):
                    ps = s_ps.tile([P, TOK], dt.float32, tag="sps")
                    for c8 in range(8):
                        nc.tensor.matmul(
                            ps[:],
                            lhsT=wk_sb[:, c8, hp * P:(hp + 1) * P],
                            rhs=xbt[:, c8, :],
                            start=(c8 == 0), stop=(c8 == 7),
                        )
                    nc.vector.tensor_scalar(
                        kT[:, hp, tb * TOK:(tb + 1) * TOK], ps[:],
                        bk_sb[:, hp:hp + 1], None, OP.add,
                    )
                for kb in range(4):
                    kcg = tb * 4 + kb
                    ps = s_ps.tile([P, 256], dt.float32, tag="sps")
                    for c8 in range(8):
                        nc.tensor.matmul(
                            ps[:],
                            lhsT=xbt[:, c8, kb * P:(kb + 1) * P],
                            rhs=wv_sb[:, c8, :],
                            start=(c8 == 0), stop=(c8 == 7),
                        )
                    nc.vector.tensor_copy(
                        v4[:, kcg, :, 0:64],
                        ps[:].rearrange("p (h w) -> p h w", w=64),
                    )
                for hp in range(2):
                    ps = s_ps.tile([P, TOK], dt.float32, tag="sps")
                    for c8 in range(8):
                        nc.tensor.matmul(
                            ps[:],
                            lhsT=wq_sb[:, c8, hp * P:(hp + 1) * P],
                            rhs=xbt[:, c8, :],
                            start=(c8 == 0), stop=(c8 == 7),
                        )
                    nc.vector.tensor_scalar(
                        qT[:, hp, tb * TOK:(tb + 1) * TOK], ps[:],
                        bq_sb[:, hp:hp + 1], None, OP.add,
                    )

            def attn_block(hp, g):
                """Causal attention for head-pair hp, query block g (512 q)."""
                nkc = 4 * g + 4
                oT = o_ps.tile([65, 2, TOK], dt.float32, tag="oT")
                for kc in range(nkc):
                    diag = kc >= 4 * g
                    qs = (kc - 4 * g) * P if diag else 0
                    sps = s_ps.tile([P, 2, TOK], dt.float32, tag="sps")
                    for h2 in range(2):
                        nc.tensor.matmul(
                            sps[:, h2, qs:],
                            lhsT=kT[64 * h2:64 * (h2 + 1), hp, kc * P:(kc + 1) * P],
                            rhs=qT[64 * h2:64 * (h2 + 1), hp,
                                   g * TOK + qs:(g + 1) * TOK],
                            start=True, stop=not diag,
                            tile_position=(64 * h2, 0),
                        )
                        if diag:
                            nc.tensor.matmul(
                                sps[:, h2, qs:qs + P],
                                lhsT=ident[:], rhs=maskm[:],
                                start=False, stop=True,
                            )
                    aT = atmp.tile([P, 2, TOK], dt.bfloat16, tag="aT")
                    nc.scalar.activation(
                        aT[:, :, qs:], sps[:, :, qs:], AF.Exp, scale=0.125
                    )
                    for h2 in range(2):
                        h = 2 * hp + h2
                        nc.tensor.matmul(
                            oT[:, h2, qs:],
                            lhsT=v_ones[:, kc, h * VW:h * VW + 65],
                            rhs=aT[:, h2, qs:],
                            start=(kc == 0), stop=(kc == nkc - 1),
                        )
                # softmax denominators -> reciprocals via ln/exp, broadcast
                lnrow = atmp2.tile([65, 2, TOK], dt.float32, tag="lnrow")
                nc.scalar.activation(lnrow[64:65, :, :], oT[64:65, :, :], AF.Ln)
                rrow = atmp2.tile([65, 2, TOK], dt.bfloat16, tag="rrow")
                nc.scalar.activation(
                    rrow[64:65, :, :], lnrow[64:65, :, :], AF.Exp, scale=-1.0
                )
                rep = s_ps.tile([P, 2, TOK], dt.float32, tag="sps")
                for h2 in range(2):
                    nc.tensor.matmul(
                        rep[0:64, h2, :],
                        lhsT=ones_hi[64:65, :], rhs=rrow[64:65, h2, :],
                        start=True, stop=True,
                    )
                rep_sb = atmp2.tile([64, 2, TOK], dt.float32, tag="rep_sb")
                nc.scalar.activation(rep_sb[:], rep[0:64, :, :], AF.Copy)
                nc.vector.tensor_tensor(
                    o_cat[hp][0:64, g * TOK:(g + 1) * TOK],
                    oT[0:64, 0, :], rep_sb[:, 0, :], OP.mult,
                )
                ot = atmp2.tile([64, TOK], dt.bfloat16, tag="o_tmp")
                nc.vector.tensor_tensor(ot[:], oT[0:64, 1, :], rep_sb[:, 1, :],
                                        OP.mult)
                nc.sync.dma_start(o_cat[hp][64:128, g * TOK:(g + 1) * TOK], ot[:])

            def wo_partial(g):
                """Own-heads partial of o @ w_o for token block g + RS."""
                stage = wop.tile([P, 8, TOK], dt.bfloat16, tag="wostage")
                for oc in range(8):
                    ps = s_ps.tile([P, TOK], dt.float32, tag="sps")
                    for ic in range(2):
                        nc.tensor.matmul(
                            ps[:],
                            lhsT=woo_sb[:, ic, oc * P:(oc + 1) * P],
                            rhs=o_cat[ic][:, g * TOK:(g + 1) * TOK],
                            start=(ic == 0), stop=(ic == 1),
                        )
                    nc.vector.tensor_copy(stage[:, oc, :], ps[:])
                rsv = rs_in[g].rearrange("(r c p t) -> r p c t", r=4, c=8, p=P)
                for r in range(4):
                    nc.sync.dma_start(
                        rsv[r], stage[:, :, r * P:(r + 1) * P])
                nc.gpsimd.collective_compute(
                    "ReduceScatter", mybir.AluOpType.add,
                    replica_groups=GROUPS,
                    ins=[rs_in[g].opt()], outs=[rs_out[g].opt()],
                )

            # ---- interleaved qkv + attention + projection schedule ----
            for g in range(NTB):
                qkv_tb(g)
                attn_block(0, g)
                attn_block(1, g)
                wo_partial(g)

            # close attention pools in LIFO order
            wop_cm.__exit__(None, None, None)
            atmp2_cm.__exit__(None, None, None)
            atmp_cm.__exit__(None, None, None)
            o_ps_cm.__exit__(None, None, None)
            s_ps_cm.__exit__(None, None, None)
            xbp_cm.__exit__(None, None, None)
            xw_cm.__exit__(None, None, None)

            # gather RS outputs: own 128-token segment of each block g
            for g in range(NTB):
                nc.sync.dma_start(
                    rsg[:, :, g * P:(g + 1) * P],
                    rs_out[g].rearrange("(c p t) -> p c t", c=8, p=P),
                )

            # ============ residual: r1 = rs + b_o + x ============
            for oc in range(8):
                nc.vector.scalar_tensor_tensor(
                    r1[:, oc, :], rsg[:, oc, :], bo_sb[:, oc:oc + 1],
                    xf[:, oc, :], op0=OP.add, op1=OP.add,
                )

            # ============ layernorm (feature-major, partition reduce) ====
            def layernorm(src, dst, g_sb, b_sb, tag):
                with (
                    tc.tile_pool(name=f"ln_{tag}", bufs=2) as lnp,
                    tc.tile_pool(name=f"lnps_{tag}", bufs=2, space="PSUM") as lnps,
                    tc.tile_pool(name=f"lnrep_{tag}", bufs=1, space="PSUM") as lnrep,
                ):
                    src_bf = lnp.tile([P, 8, TOK], dt.bfloat16, tag=f"srcbf_{tag}")
                    for c8 in range(8):
                        nc.vector.tensor_copy(src_bf[:, c8, :], src[:, c8, :])
                    s1 = lnps.tile([1, TOK], dt.float32, tag=f"s1_{tag}")
                    s2 = lnps.tile([1, TOK], dt.float32, tag=f"s2_{tag}")
                    for c8 in range(8):
                        sq = lnp.tile([P, TOK], dt.bfloat16, tag=f"sq_{tag}")
                        nc.vector.tensor_tensor(sq[:], src[:, c8, :], src[:, c8, :], OP.mult)
                        nc.tensor.matmul(
                            s1[:], lhsT=ones128_bf[:], rhs=src_bf[:, c8, :],
                            start=(c8 == 0), stop=(c8 == 7),
                        )
                        nc.tensor.matmul(
                            s2[:], lhsT=ones128_bf[:], rhs=sq[:],
                            start=(c8 == 0), stop=(c8 == 7),
                        )
                    mu = lnp.tile([1, TOK], dt.float32, tag=f"mu_{tag}")
                    nc.vector.tensor_scalar(mu[:], s1[:], 1.0 / D, None, OP.mult)
                    m2 = lnp.tile([1, TOK], dt.float32, tag=f"m2_{tag}")
                    nc.vector.tensor_scalar(m2[:], s2[:], 1.0 / D, LN_EPS, OP.mult, OP.add)
                    var = lnp.tile([1, TOK], dt.float32, tag=f"var_{tag}")
                    nc.vector.tensor_tensor(var[:], mu[:], mu[:], OP.mult)
                    nc.vector.tensor_tensor(var[:], m2[:], var[:], OP.subtract)
                    lnv = lnp.tile([1, TOK], dt.float32, tag=f"lnv_{tag}")
                    nc.scalar.activation(lnv[:], var[:], AF.Ln)
                    rsmu = lnp.tile([1, 2, TOK], dt.float32, tag=f"rsmu_{tag}")
                    nc.scalar.activation(rsmu[:, 0, :], lnv[:], AF.Exp, scale=-0.5)
                    nc.vector.tensor_tensor(
                        rsmu[:, 1, :], mu[:], rsmu[:, 0, :], OP.mult
                    )
                    rep = lnrep.tile([P, 2, TOK], dt.float32, tag=f"rep_{tag}")
                    for i in range(2):
                        nc.tensor.matmul(
                            rep[:, i, :], lhsT=ones1_f[:], rhs=rsmu[:, i, :],
                            start=True, stop=True,
                        )
                    rep_sb = lnp.tile([P, 2, TOK], dt.float32, tag=f"repsb_{tag}")
                    nc.scalar.activation(rep_sb[:], rep[:], AF.Copy)
                    for c8 in range(8):
                        t = lnp.tile([P, TOK], dt.float32, tag=f"t_{tag}")
                        nc.vector.tensor_tensor(
                            t[:], src[:, c8, :], rep_sb[:, 0, :], OP.mult
                        )
                        nc.vector.tensor_tensor(t[:], t[:], rep_sb[:, 1, :], OP.subtract)
                        nc.vector.tensor_scalar(
                            dst[:, c8, :], t[:], g_sb[:, c8:c8 + 1], b_sb[:, c8:c8 + 1],
                            OP.mult, OP.add,
                        )

            pA_cm.__exit__(None, None, None)
            pN_cm = tc.tile_pool(name="pN", bufs=1)
            pN = pN_cm.__enter__()
            n_sb = pN.tile([P, 8, TOK], dt.float32, tag="n_sb")
            n_bf = [pN.tile([P, TOK], dt.bfloat16, tag=f"n_bf{i}", name=f"n_bf{i}")
                    for i in range(8)]
            h1g = pN.tile([P, 32, TOK], dt.bfloat16, tag="h1g")
            r2 = pN.tile([P, 8, TOK], dt.float32, tag="r2")

            # MLP weight pools + prefetches (DMAs overlap LN1)
            wfcp_cm = tc.tile_pool(name="wfc", bufs=2)
            wfcp = wfcp_cm.__enter__()

            def load_wfc(oq):
                wq_ = wfcp.tile([P, 8, 1024], dt.bfloat16, tag="wfcq")
                nc.sync.dma_start(
                    wq_[:],
                    w_fc[:, oq * 1024:(oq + 1) * 1024].rearrange(
                        "(c p) f -> p c f", p=P),
                )
                return wq_

            wprp_cm = tc.tile_pool(name="wpr", bufs=2)
            wprp = wprp_cm.__enter__()

            def load_wpr(q4):
                wq_ = wprp.tile([P, 8, D], dt.bfloat16, tag="wprq")
                nc.sync.dma_start(
                    wq_[:],
                    w_pr[q4 * 1024:(q4 + 1) * 1024, :].rearrange(
                        "(c p) f -> p c f", p=P),
                )
                return wq_

            wfc_q0 = load_wfc(0)
            wpr_q0 = load_wpr(0)  # prefetch during fc

            layernorm(r1, n_sb, g1_sb, b1_sb, "ln1")
            for c8 in range(8):
                nc.vector.tensor_copy(n_bf[c8][:], n_sb[:, c8, :])

            with tc.tile_pool(name="fc_ps", bufs=4, space="PSUM") as fc_ps:
                for oq in range(4):
                    wq_ = wfc_q0 if oq == 0 else load_wfc(oq)
                    for oc8 in range(8):
                        oc = oq * 8 + oc8
                        ps = fc_ps.tile([P, TOK], dt.float32, tag="fcps")
                        for c8 in range(8):
                            nc.tensor.matmul(
                                ps[:],
                                lhsT=wq_[:, c8, oc8 * P:(oc8 + 1) * P],
                                rhs=n_bf[c8][:],
                                start=(c8 == 0), stop=(c8 == 7),
                            )
                        nc.scalar.activation(
                            h1g[:, oc, :], ps[:], AF.Gelu,
                            bias=bfc_sb[:, oc:oc + 1],
                        )

            with tc.tile_pool(name="pr_ps", bufs=1, space="PSUM") as pr_ps:
                mps = [pr_ps.tile([P, TOK], dt.float32, tag=f"mps{i}", name=f"mps{i}")
                       for i in range(8)]
                for q4 in range(4):
                    wq_ = wpr_q0 if q4 == 0 else load_wpr(q4)
                    for oc in range(8):
                        for c8 in range(8):
                            nc.tensor.matmul(
                                mps[oc][:],
                                lhsT=wq_[:, c8, oc * P:(oc + 1) * P],
                                rhs=h1g[:, q4 * 8 + c8, :],
                                start=(q4 == 0 and c8 == 0),
                                stop=(q4 == 3 and c8 == 7),
                            )
                for oc in range(8):
                    nc.vector.scalar_tensor_tensor(
                        r2[:, oc, :], mps[oc][:], bpr_sb[:, oc:oc + 1], n_sb[:, oc, :],
                        op0=OP.add, op1=OP.add,
                    )
            wprp_cm.__exit__(None, None, None)
            wfcp_cm.__exit__(None, None, None)

            layernorm(r2, r2, g2_sb, b2_sb, "ln2")
            out_v = out_d.rearrange("(c p) t -> p c t", p=P)
            for c8 in range(8):
                nc.sync.dma_start(out_v[:, c8, :], r2[:, c8, :])
            pN_cm.__exit__(None, None, None)
            pR_cm.__exit__(None, None, None)

    nc.compile()
    return nc


def _prep_shared(w_attn, b_attn, w_o, b_o, ln1_g, ln1_b, w_fc, b_fc, w_pr, b_pr,
                 ln2_g, ln2_b):
    w_attn = np.asarray(w_attn, np.float32)
    b_attn = np.asarray(b_attn, np.float32)
    w_o_f = np.asarray(w_o, np.float32)
    b_v = b_attn[2 * D:]
    b_o_eff = (np.asarray(b_o, np.float32) + b_v @ w_o_f).astype(np.float32)
    mask = np.where(
        np.arange(P)[:, None] > np.arange(P)[None, :], MASK_NEG, 0.0
    ).astype(BF16)  # [ki, qj]: mask keys above the diagonal
    shared = {
        "w_fc": np.asarray(w_fc, np.float32).astype(BF16),
        "w_pr": np.asarray(w_pr, np.float32).astype(BF16),
        "b_o": b_o_eff,
        "b_fc": np.asarray(b_fc, np.float32),
        "b_pr": np.asarray(b_pr, np.float32),
        "g1": np.asarray(ln1_g, np.float32),
        "b1": np.asarray(ln1_b, np.float32),
        "g2": np.asarray(ln2_g, np.float32),
        "b2": np.asarray(ln2_b, np.float32),
        "ident": np.eye(P, dtype=np.float32).astype(BF16),
        "maskm": mask,
    }
    return shared, w_attn, b_attn, w_o_f


def _own_token_idx(j):
    """RS-order tokens of core 4b+j: segment j of each 512-block."""
    return np.concatenate(
        [np.arange(TOK * g + P * j, TOK * g + P * (j + 1)) for g in range(NTB)]
    )


def kernel(x, w_attn, b_attn, w_o, b_o, ln1_g, ln1_b, w_fc, b_fc, w_pr, b_pr,
           ln2_g, ln2_b, _trace=False):
    from concourse.bass_utils import run_bass_kernel_spmd

    if "nc" not in _CACHE:
        _CACHE["nc"] = _build()
    nc = _CACHE["nc"]

    x = np.asarray(x, np.float32)
    shared, w_attn_f, b_attn_f, w_o_f = _prep_shared(
        w_attn, b_attn, w_o, b_o, ln1_g, ln1_b, w_fc, b_fc, w_pr, b_pr,
        ln2_g, ln2_b)

    xTb_bf = [np.ascontiguousarray(x[b].T).astype(BF16) for b in range(B)]

    in_maps = []
    idxs = []
    for c in range(N_CORES):
        b, j = c // 4, c % 4
        idx = _own_token_idx(j)
        idxs.append((b, idx))
        m = dict(shared)
        m["xTb"] = xTb_bf[b]
        m["xTf"] = np.ascontiguousarray(x[b, idx, :].T)
        m["w_q"] = np.ascontiguousarray(
            w_attn_f[:, 256 * j:256 * (j + 1)]).astype(BF16)
        m["w_k"] = np.ascontiguousarray(
            w_attn_f[:, D + 256 * j:D + 256 * (j + 1)]).astype(BF16)
        m["w_v"] = np.ascontiguousarray(
            w_attn_f[:, 2 * D + 256 * j:2 * D + 256 * (j + 1)]).astype(BF16)
        m["w_oo"] = np.ascontiguousarray(
            w_o_f[256 * j:256 * (j + 1), :]).astype(BF16)
        m["b_q"] = np.ascontiguousarray(b_attn_f[256 * j:256 * (j + 1)])
        m["b_k"] = np.ascontiguousarray(b_attn_f[D + 256 * j:D + 256 * (j + 1)])
        in_maps.append(m)

    res = run_bass_kernel_spmd(
        nc, in_maps, core_ids=list(range(N_CORES)), trace=_trace
    )
    if _trace:
        _CACHE["exec_time_ns"] = res.exec_time_ns
        _CACHE["insts_and_trace"] = res.instructions_and_trace

    out = np.empty((B, S, D), np.float32)
    for c in range(N_CORES):
        b, idx = idxs[c]
        out[b, idx, :] = res.results[c]["out"].T
    return out


# revision 21
# speedup vs baseline: 1.1505x; 1.0388x over previous
"""Dense transformer block (attention + post-LN MLP) on 8 trn2 NeuronCores.

Head-parallel sharding: core c = 4b+j handles heads {4j..4j+3} of batch
b over the full 2048-token sequence, so every core runs the same
uniform causal-attention program (no runtime Switch) and there is no
K/V exchange: each core computes q/k/v for its own heads over all
tokens. Each core also computes its heads' partial contribution to the
attention projection (o @ w_o rows) for ALL tokens; four pipelined
ReduceScatters (one per 512-token block, issued as each block's
attention completes) sum the partials across the 4-core batch group and
deliver each core its own 128-token segment — at a core-independent
address. The MLP is token-sharded over the RS-assigned tokens. Weights
replicated (bf16); activations feature-major (x^T) end-to-end. qkv /
attention / softmax-exp / w_o-partial work is interleaved in emission
order so the scalar-engine exp stream hides under tensor-engine
matmuls.
"""

import numpy as np
import ml_dtypes

BF16 = ml_dtypes.bfloat16

N_CORES = 8
B, S, D = 2, 2048, 1024
H, HD = 16, 64
F = 4 * D
TOK = 512            # tokens owned per core (MLP/output shard)
P = 128
NTB = 4              # 512-token blocks per sequence
VW = 66              # per-head V row width: 64 v + ones col + pad
MASK_NEG = -80000.0  # -> -79872 in bf16; /8 => exp underflows to exactly 0
LN_EPS = 1e-5

_CACHE = {}


def _patched_insert_act_table_loads(self):
    """Instance-level replacement for Bacc.insert_act_table_loads that
    removes Exp/Ln from every table set except natural_log_exp_and_others,
    so the softmax reciprocal's Ln/Exp alternation resolves to ONE set and
    the per-head ACT_TABLE_LOAD thrash (~1.3us each) disappears."""
    import bass_rust as _bass_rust
    import concourse.mybir as mybir
    from concourse.hw_specs import get_activation_tables

    has_activation = any(
        isinstance(i, mybir.InstActivation)
        for b in self.main_func.blocks
        for i in b.instructions
    )
    if not has_activation:
        return
    tabs = get_activation_tables(self.m.arch)
    AF = mybir.ActivationFunctionType
    if "natural_log_exp_and_others" in tabs:
        for name, fns in tabs.items():
            if name != "natural_log_exp_and_others":
                fns.discard(AF.Exp)
                fns.discard(AF.Ln)
    _bass_rust.insert_act_table_loads(self, list(tabs.items()))


def _build():
    import types

    import concourse.bass as bass
    import concourse.mybir as mybir
    import concourse.tile as tile
    from concourse import bacc

    dt = mybir.dt
    AF = mybir.ActivationFunctionType
    OP = mybir.AluOpType

    nc = bacc.Bacc(
        "TRN2",
        target_bir_lowering=False,
        debug=False,
        enable_asserts=True,
        num_devices=N_CORES,
    )
    try:
        nc.insert_act_table_loads = types.MethodType(
            _patched_insert_act_table_loads, nc
        )
    except Exception:
        pass

    def din(name, shape, dty):
        return nc.dram_tensor(name, shape, dty, kind="ExternalInput").ap()

    xTb = din("xTb", [D, S], dt.bfloat16)       # own batch, feature-major
    xTf = din("xTf", [D, TOK], dt.float32)      # own (RS-order) tokens
    w_q = din("w_q", [D, 256], dt.bfloat16)     # own 4 heads
    w_k = din("w_k", [D, 256], dt.bfloat16)
    w_v = din("w_v", [D, 256], dt.bfloat16)
    w_oo = din("w_oo", [256, D], dt.bfloat16)   # w_o rows of own heads
    w_fc = din("w_fc", [D, F], dt.bfloat16)
    w_pr = din("w_pr", [F, D], dt.bfloat16)
    b_q = din("b_q", [256], dt.float32)
    b_k = din("b_k", [256], dt.float32)
    b_o = din("b_o", [D], dt.float32)           # b_o_eff (v-bias folded)
    b_fc = din("b_fc", [F], dt.float32)
    b_pr = din("b_pr", [D], dt.float32)
    g1 = din("g1", [D], dt.float32)
    b1 = din("b1", [D], dt.float32)
    g2 = din("g2", [D], dt.float32)
    b2 = din("b2", [D], dt.float32)
    ident_d = din("ident", [P, P], dt.bfloat16)
    maskm_d = din("maskm", [P, P], dt.bfloat16)
    out_d = nc.dram_tensor("out", [D, TOK], dt.float32, kind="ExternalOutput").ap()

    GROUPS = [[0, 1, 2, 3], [4, 5, 6, 7]]

    with tile.TileContext(nc) as tc:
        from contextlib import ExitStack

        ctx = ExitStack()
        with ctx:
            c_pool = ctx.enter_context(tc.tile_pool(name="consts", bufs=1))
            dram = ctx.enter_context(tc.tile_pool(name="dram", bufs=1, space="DRAM"))

            # ---- constants ----
            ident = c_pool.tile([P, P], dt.bfloat16, tag="ident")
            nc.sync.dma_start(ident[:], ident_d[:])
            maskm = c_pool.tile([P, P], dt.bfloat16, tag="maskm")
            nc.sync.dma_start(maskm[:], maskm_d[:])
            ones128_bf = c_pool.tile([P, 1], dt.bfloat16, tag="ones128")
            nc.vector.memset(ones128_bf[:], 1.0)
            ones1_f = c_pool.tile([1, P], dt.float32, tag="ones1f")
            nc.vector.memset(ones1_f[:], 1.0)
            ones_hi = c_pool.tile([65, 64], dt.bfloat16, tag="oneshi")
            nc.vector.memset(ones_hi[64:65, :], 1.0)

            bq_sb = c_pool.tile([P, 2], dt.float32, tag="bq")
            nc.sync.dma_start(bq_sb[:], b_q.rearrange("(c p) -> p c", p=P))
            bk_sb = c_pool.tile([P, 2], dt.float32, tag="bk")
            nc.sync.dma_start(bk_sb[:], b_k.rearrange("(c p) -> p c", p=P))
            bo_sb = c_pool.tile([P, 8], dt.float32, tag="bo")
            nc.sync.dma_start(bo_sb[:], b_o.rearrange("(c p) -> p c", p=P))
            bfc_sb = c_pool.tile([P, 32], dt.float32, tag="bfc")
            nc.sync.dma_start(bfc_sb[:], b_fc.rearrange("(c p) -> p c", p=P))
            bpr_sb = c_pool.tile([P, 8], dt.float32, tag="bpr")
            nc.sync.dma_start(bpr_sb[:], b_pr.rearrange("(c p) -> p c", p=P))
            g1_sb = c_pool.tile([P, 8], dt.float32, tag="g1")
            nc.sync.dma_start(g1_sb[:], g1.rearrange("(c p) -> p c", p=P))
            b1_sb = c_pool.tile([P, 8], dt.float32, tag="b1")
            nc.sync.dma_start(b1_sb[:], b1.rearrange("(c p) -> p c", p=P))
            g2_sb = c_pool.tile([P, 8], dt.float32, tag="g2")
            nc.sync.dma_start(g2_sb[:], g2.rearrange("(c p) -> p c", p=P))
            b2_sb = c_pool.tile([P, 8], dt.float32, tag="b2")
            nc.sync.dma_start(b2_sb[:], b2.rearrange("(c p) -> p c", p=P))

            # ---- ReduceScatter buffers: one per 512-token block ----
            rs_in = [dram.tile([4 * D * P], dt.bfloat16, tag=f"rsi{g}",
                               name=f"rsi{g}") for g in range(NTB)]
            rs_out = [dram.tile([D * P], dt.bfloat16, tag=f"rso{g}",
                                name=f"rso{g}") for g in range(NTB)]

            # tiny warm-up collective: absorbs the runtime's first-
            # collective global barrier while qkv is still starting up,
            # so the pipelined RS chain below isn't delayed behind it.
            wu_in = dram.tile([256], dt.bfloat16, tag="wui", name="wui")
            wu_out = dram.tile([4, 256], dt.bfloat16, tag="wuo", name="wuo")
            nc.gpsimd.collective_compute(
                "AllGather", mybir.AluOpType.bypass,
                replica_groups=GROUPS,
                ins=[wu_in.opt()], outs=[wu_out.opt()],
            )

            # ---- long-lived activations ----
            pR_cm = tc.tile_pool(name="pR", bufs=1)
            pR = pR_cm.__enter__()
            r1 = pR.tile([P, 8, TOK], dt.float32, tag="r1")
            xf = pR.tile([P, 8, TOK], dt.float32, tag="xf")
            rsg = pR.tile([P, 8, TOK], dt.bfloat16, tag="rsg")

            pA_cm = tc.tile_pool(name="pA", bufs=1)
            pA = pA_cm.__enter__()
            qT = pA.tile([P, 2, S], dt.bfloat16, tag="qT")
            kT = pA.tile([P, 2, S], dt.bfloat16, tag="kT")
            v_ones = pA.tile([P, 16, 4 * VW], dt.bfloat16, tag="v_ones")
            o_cat = [pA.tile([P, S], dt.bfloat16, tag=f"o_cat{i}", name=f"o_cat{i}")
                     for i in range(2)]
            woo_sb = pA.tile([P, 2, D], dt.bfloat16, tag="woo")

            v4 = v_ones.rearrange("p k (h w) -> p k h w", w=VW)
            nc.vector.memset(v4[:, :, :, 64:66], 0.0)
            nc.vector.memset(v4[:, :, :, 64:65], 1.0)

            # ---- qkv inputs ----
            xw_cm = tc.tile_pool(name="xw", bufs=1)
            xw = xw_cm.__enter__()
            wk_sb = xw.tile([P, 8, 256], dt.bfloat16, tag="wk")
            nc.sync.dma_start(wk_sb[:], w_k.rearrange("(c p) f -> p c f", p=P))
            wq_sb = xw.tile([P, 8, 256], dt.bfloat16, tag="wq")
            nc.sync.dma_start(wq_sb[:], w_q.rearrange("(c p) f -> p c f", p=P))
            wv_sb = xw.tile([P, 8, 256], dt.bfloat16, tag="wv")
            nc.sync.dma_start(wv_sb[:], w_v.rearrange("(c p) f -> p c f", p=P))
            nc.sync.dma_start(woo_sb[:], w_oo.rearrange("(c p) f -> p c f", p=P))
            xbp_cm = tc.tile_pool(name="xbp", bufs=3)
            xbp = xbp_cm.__enter__()
            xbv = xTb.rearrange("(c p) t -> p c t", p=P)

            # prefetch for the post-attention phases
            nc.sync.dma_start(xf[:], xTf.rearrange("(c p) t -> p c t", p=P))

            # ---- attention working pools ----
            # qkv/w_o psum shares the scores pool ("sps" tag): 2 slots x
            # 2 banks + oT 2 slots x 2 banks = all 8 banks.
            s_ps_cm = tc.tile_pool(name="att_s", bufs=2, space="PSUM")
            s_ps = s_ps_cm.__enter__()
            o_ps_cm = tc.tile_pool(name="att_o", bufs=2, space="PSUM")
            o_ps = o_ps_cm.__enter__()
            atmp_cm = tc.tile_pool(name="att_tmp", bufs=3)
            atmp = atmp_cm.__enter__()
            atmp2_cm = tc.tile_pool(name="att_tmp2", bufs=2)
            atmp2 = atmp2_cm.__enter__()
            wop_cm = tc.tile_pool(name="wop", bufs=2)
            wop = wop_cm.__enter__()

            def qkv_tb(tb):
                """q/k/v for token block tb (512 tokens), own 4 heads."""
                xbt = xbp.tile([P, 8, TOK], dt.bfloat16, tag="xb")
                nc.sync.dma_start(
                    xbt[:], xbv[:, :, tb * TOK:(tb + 1) * TOK])
                for hp in range(2):
                    ps = s_ps.tile([P, TOK], dt.float32, tag="sps")
                    for c8 in range(8):
                        nc.tensor.matmul(
                            ps[:],
                            lhsT=wk_sb[:, c8, hp * P:(hp + 1) * P],
                            rhs=xbt[:, c8, :],
                            start=(c8 == 0), stop=(c8 == 7),
                        )
                    nc.vector.tensor_scalar(
                        kT[:, hp, tb * TOK:(tb + 1) * TOK], ps[:],
                        bk_sb[:, hp:hp + 1], None, OP.add,
                    )
                for kb in range(4):
                    kcg = tb * 4 + kb
                    ps = s_ps.tile([P, 256], dt.float32, tag="sps")
                    for c8 in range(8):
                        nc.tensor.matmul(
                            ps[:],
                            lhsT=xbt[:, c8, kb * P:(kb + 1) * P],
                            rhs=wv_sb[:, c8, :],
                            start=(c8 == 0), stop=(c8 == 7),
                        )
                    nc.vector.tensor_copy(
                        v4[:, kcg, :, 0:64],
                        ps[:].rearrange("p (h w) -> p h w", w=64),
                    )
                for hp in range(2):
                    ps = s_ps.tile([P, TOK], dt.float32, tag="sps")
                    for c8 in range(8):
                        nc.tensor.matmul(
                            ps[:],
                            lhsT=wq_sb[:, c8, hp * P:(hp + 1) * P],
                            rhs=xbt[:, c8, :],
                            start=(c8 == 0), stop=(c8 == 7),
                        )
                    nc.vector.tensor_scalar(
                        qT[:, hp, tb * TOK:(tb + 1) * TOK], ps[:],
                        bq_sb[:, hp:hp + 1], None, OP.add,
                    )

            def attn_core(hp, g):
                """Causal attention for head-pair hp, query block g (512 q).
                AV matmuls are emitted one key-chunk behind the scores so
                the PE never stalls on the scalar-engine exp."""
                nkc = 4 * g + 4
                oT = o_ps.tile([65, 2, TOK], dt.float32, tag="oT")
                pend = None

                def emit_av(item):
                    aT_, kc_ = item
                    diag_ = kc_ >= 4 * g
                    qs_ = (kc_ - 4 * g) * P if diag_ else 0
                    for h2 in range(2):
                        h = 2 * hp + h2
                        nc.tensor.matmul(
                            oT[:, h2, qs_:],
                            lhsT=v_ones[:, kc_, h * VW:h * VW + 65],
                            rhs=aT_[:, h2, qs_:],
                            start=(kc_ == 0), stop=(kc_ == nkc - 1),
                        )

                for kc in range(nkc):
                    diag = kc >= 4 * g
                    qs = (kc - 4 * g) * P if diag else 0
                    sps = s_ps.tile([P, 2, TOK], dt.float32, tag="sps")
                    for h2 in range(2):
                        nc.tensor.matmul(
                            sps[:, h2, qs:],
                            lhsT=kT[64 * h2:64 * (h2 + 1), hp, kc * P:(kc + 1) * P],
                            rhs=qT[64 * h2:64 * (h2 + 1), hp,
                                   g * TOK + qs:(g + 1) * TOK],
                            start=True, stop=not diag,
                            tile_position=(64 * h2, 0),
                        )
                        if diag:
                            nc.tensor.matmul(
                                sps[:, h2, qs:qs + P],
                                lhsT=ident[:], rhs=maskm[:],
                                start=False, stop=True,
                            )
                    aT = atmp.tile([P, 2, TOK], dt.bfloat16, tag="aT")
                    nc.scalar.activation(
                        aT[:, :, qs:], sps[:, :, qs:], AF.Exp, scale=0.125
                    )
                    if pend is not None:
                        emit_av(pend)
                    pend = (aT, kc)
                emit_av(pend)
                return oT

            def attn_norm(hp, g, oT):
                # softmax denominators -> reciprocals via ln/exp, broadcast
                lnrow = atmp2.tile([65, 2, TOK], dt.float32, tag="lnrow")
                nc.scalar.activation(lnrow[64:65, :, :], oT[64:65, :, :], AF.Ln)
                rrow = atmp2.tile([65, 2, TOK], dt.bfloat16, tag="rrow")
                nc.scalar.activation(
                    rrow[64:65, :, :], lnrow[64:65, :, :], AF.Exp, scale=-1.0
                )
                rep = s_ps.tile([P, 2, TOK], dt.float32, tag="sps")
                for h2 in range(2):
                    nc.tensor.matmul(
                        rep[0:64, h2, :],
                        lhsT=ones_hi[64:65, :], rhs=rrow[64:65, h2, :],
                        start=True, stop=True,
                    )
                rep_sb = atmp2.tile([64, 2, TOK], dt.float32, tag="rep_sb")
                nc.scalar.activation(rep_sb[:], rep[0:64, :, :], AF.Copy)
                nc.vector.tensor_tensor(
                    o_cat[hp][0:64, g * TOK:(g + 1) * TOK],
                    oT[0:64, 0, :], rep_sb[:, 0, :], OP.mult,
                )
                ot = atmp2.tile([64, TOK], dt.bfloat16, tag="o_tmp")
                nc.vector.tensor_tensor(ot[:], oT[0:64, 1, :], rep_sb[:, 1, :],
                                        OP.mult)
                nc.sync.dma_start(o_cat[hp][64:128, g * TOK:(g + 1) * TOK], ot[:])

            def wo_partial(g):
                """Own-heads partial of o @ w_o for token block g + RS."""
                stage = wop.tile([P, 8, TOK], dt.bfloat16, tag="wostage")
                for oc in range(8):
                    ps = s_ps.tile([P, TOK], dt.float32, tag="sps")
                    for ic in range(2):
                        nc.tensor.matmul(
                            ps[:],
                            lhsT=woo_sb[:, ic, oc * P:(oc + 1) * P],
                            rhs=o_cat[ic][:, g * TOK:(g + 1) * TOK],
                            start=(ic == 0), stop=(ic == 1),
                        )
                    nc.vector.tensor_copy(stage[:, oc, :], ps[:])
                rsv = rs_in[g].rearrange("(r c p t) -> r p c t", r=4, c=8, p=P)
                for r in range(4):
                    nc.sync.dma_start(
                        rsv[r], stage[:, :, r * P:(r + 1) * P])
                nc.gpsimd.collective_compute(
                    "ReduceScatter", mybir.AluOpType.add,
                    replica_groups=GROUPS,
                    ins=[rs_in[g].opt()], outs=[rs_out[g].opt()],
                )

            # ---- interleaved qkv + attention + projection schedule ----
            # norms of block g are deferred past qkv(g+1) so their ACT
            # chain overlaps qkv matmuls instead of stalling the PE.
            qkv_tb(0)
            o00 = attn_core(0, 0)
            o10 = attn_core(1, 0)
            prev = (o00, o10)
            for g in range(1, NTB):
                qkv_tb(g)
                attn_norm(0, g - 1, prev[0])
                attn_norm(1, g - 1, prev[1])
                wo_partial(g - 1)
                prev = (attn_core(0, g), attn_core(1, g))
            attn_norm(0, NTB - 1, prev[0])
            attn_norm(1, NTB - 1, prev[1])
            wo_partial(NTB - 1)

            # close attention pools in LIFO order
            wop_cm.__exit__(None, None, None)
            atmp2_cm.__exit__(None, None, None)
            atmp_cm.__exit__(None, None, None)
            o_ps_cm.__exit__(None, None, None)
            s_ps_cm.__exit__(None, None, None)
            xbp_cm.__exit__(None, None, None)
            xw_cm.__exit__(None, None, None)

            # gather RS outputs: own 128-token segment of each block g
            for g in range(NTB):
                nc.sync.dma_start(
                    rsg[:, :, g * P:(g + 1) * P],
                    rs_out[g].rearrange("(c p t) -> p c t", c=8, p=P),
                )

            # ============ layernorm (feature-major, partition reduce) ====
            # operates on a column (token) subrange so the tail can be
            # pipelined against the last ReduceScatter.
            def layernorm(src, dst, g_sb, b_sb, tag, lo, wd):
                with (
                    tc.tile_pool(name=f"ln_{tag}", bufs=2) as lnp,
                    tc.tile_pool(name=f"lnps_{tag}", bufs=2, space="PSUM") as lnps,
                    tc.tile_pool(name=f"lnrep_{tag}", bufs=1, space="PSUM") as lnrep,
                ):
                    src_bf = lnp.tile([P, 8, wd], dt.bfloat16, tag=f"srcbf_{tag}")
                    for c8 in range(8):
                        nc.vector.tensor_copy(src_bf[:, c8, :], src[:, c8, lo:lo + wd])
                    s1 = lnps.tile([1, wd], dt.float32, tag=f"s1_{tag}")
                    s2 = lnps.tile([1, wd], dt.float32, tag=f"s2_{tag}")
                    for c8 in range(8):
                        sq = lnp.tile([P, wd], dt.bfloat16, tag=f"sq_{tag}")
                        nc.vector.tensor_tensor(sq[:], src[:, c8, lo:lo + wd],
                                                src[:, c8, lo:lo + wd], OP.mult)
                        nc.tensor.matmul(
                            s1[:], lhsT=ones128_bf[:], rhs=src_bf[:, c8, :],
                            start=(c8 == 0), stop=(c8 == 7),
                        )
                        nc.tensor.matmul(
                            s2[:], lhsT=ones128_bf[:], rhs=sq[:],
                            start=(c8 == 0), stop=(c8 == 7),
                        )
                    mu = lnp.tile([1, wd], dt.float32, tag=f"mu_{tag}")
                    nc.vector.tensor_scalar(mu[:], s1[:], 1.0 / D, None, OP.mult)
                    m2 = lnp.tile([1, wd], dt.float32, tag=f"m2_{tag}")
                    nc.vector.tensor_scalar(m2[:], s2[:], 1.0 / D, LN_EPS, OP.mult, OP.add)
                    var = lnp.tile([1, wd], dt.float32, tag=f"var_{tag}")
                    nc.vector.tensor_tensor(var[:], mu[:], mu[:], OP.mult)
                    nc.vector.tensor_tensor(var[:], m2[:], var[:], OP.subtract)
                    lnv = lnp.tile([1, wd], dt.float32, tag=f"lnv_{tag}")
                    nc.scalar.activation(lnv[:], var[:], AF.Ln)
                    rsmu = lnp.tile([1, 2, wd], dt.float32, tag=f"rsmu_{tag}")
                    nc.scalar.activation(rsmu[:, 0, :], lnv[:], AF.Exp, scale=-0.5)
                    nc.vector.tensor_tensor(
                        rsmu[:, 1, :], mu[:], rsmu[:, 0, :], OP.mult
                    )
                    rep = lnrep.tile([P, 2, wd], dt.float32, tag=f"rep_{tag}")
                    for i in range(2):
                        nc.tensor.matmul(
                            rep[:, i, :], lhsT=ones1_f[:], rhs=rsmu[:, i, :],
                            start=True, stop=True,
                        )
                    rep_sb = lnp.tile([P, 2, wd], dt.float32, tag=f"repsb_{tag}")
                    nc.scalar.activation(rep_sb[:], rep[:], AF.Copy)
                    for c8 in range(8):
                        t = lnp.tile([P, wd], dt.float32, tag=f"t_{tag}")
                        nc.vector.tensor_tensor(
                            t[:], src[:, c8, lo:lo + wd], rep_sb[:, 0, :], OP.mult
                        )
                        nc.vector.tensor_tensor(t[:], t[:], rep_sb[:, 1, :], OP.subtract)
                        nc.vector.tensor_scalar(
                            dst[:, c8, lo:lo + wd], t[:], g_sb[:, c8:c8 + 1],
                            b_sb[:, c8:c8 + 1], OP.mult, OP.add,
                        )

            pA_cm.__exit__(None, None, None)
            pN_cm = tc.tile_pool(name="pN", bufs=1)
            pN = pN_cm.__enter__()
            n_sb = pN.tile([P, 8, TOK], dt.float32, tag="n_sb")
            n_bf = [pN.tile([P, TOK], dt.bfloat16, tag=f"n_bf{i}", name=f"n_bf{i}")
                    for i in range(8)]
            h1g = pN.tile([P, 32, TOK], dt.bfloat16, tag="h1g")
            r2 = r1  # reuse: each column range of r1 is dead before r2 writes it

            # MLP weight pools + prefetch
            wfcp_cm = tc.tile_pool(name="wfc", bufs=2)
            wfcp = wfcp_cm.__enter__()

            def load_wfc(oq):
                wq_ = wfcp.tile([P, 8, 1024], dt.bfloat16, tag="wfcq")
                nc.sync.dma_start(
                    wq_[:],
                    w_fc[:, oq * 1024:(oq + 1) * 1024].rearrange(
                        "(c p) f -> p c f", p=P),
                )
                return wq_

            wprp_cm = tc.tile_pool(name="wpr", bufs=2)
            wprp = wprp_cm.__enter__()

            def load_wpr(q4):
                wq_ = wprp.tile([P, 8, D], dt.bfloat16, tag="wprq")
                nc.sync.dma_start(
                    wq_[:],
                    w_pr[q4 * 1024:(q4 + 1) * 1024, :].rearrange(
                        "(c p) f -> p c f", p=P),
                )
                return wq_

            wfc_q0 = load_wfc(0)

            # ==== tail, pipelined in two 256-token column halves:
            # half 0 (RS blocks 0,1) starts while RS(2)/RS(3) still fly.
            HW_ = TOK // 2
            for hh in range(2):
                lo = HW_ * hh
                # residual: r1 = rs + b_o + x
                for oc in range(8):
                    nc.vector.scalar_tensor_tensor(
                        r1[:, oc, lo:lo + HW_], rsg[:, oc, lo:lo + HW_],
                        bo_sb[:, oc:oc + 1], xf[:, oc, lo:lo + HW_],
                        op0=OP.add, op1=OP.add,
                    )
                layernorm(r1, n_sb, g1_sb, b1_sb, f"ln1{hh}", lo, HW_)
                for c8 in range(8):
                    nc.vector.tensor_copy(n_bf[c8][:, lo:lo + HW_],
                                          n_sb[:, c8, lo:lo + HW_])
                with tc.tile_pool(name=f"fc_ps{hh}", bufs=4, space="PSUM") as fc_ps:
                    for oq in range(4):
                        wq_ = wfc_q0 if (hh == 0 and oq == 0) else load_wfc(oq)
                        for oc8 in range(8):
                            oc = oq * 8 + oc8
                            ps = fc_ps.tile([P, HW_], dt.float32, tag="fcps")
                            for c8 in range(8):
                                nc.tensor.matmul(
                                    ps[:],
                                    lhsT=wq_[:, c8, oc8 * P:(oc8 + 1) * P],
                                    rhs=n_bf[c8][:, lo:lo + HW_],
                                    start=(c8 == 0), stop=(c8 == 7),
                                )
                            nc.scalar.activation(
                                h1g[:, oc, lo:lo + HW_], ps[:], AF.Gelu,
                                bias=bfc_sb[:, oc:oc + 1],
                            )
                with tc.tile_pool(name=f"pr_ps{hh}", bufs=1, space="PSUM") as pr_ps:
                    mps = [pr_ps.tile([P, HW_], dt.float32, tag=f"mps{i}",
                                      name=f"mps{hh}_{i}") for i in range(8)]
                    for q4 in range(4):
                        wq_ = load_wpr(q4)
                        for oc in range(8):
                            for c8 in range(8):
                                nc.tensor.matmul(
                                    mps[oc][:],
                                    lhsT=wq_[:, c8, oc * P:(oc + 1) * P],
                                    rhs=h1g[:, q4 * 8 + c8, lo:lo + HW_],
                                    start=(q4 == 0 and c8 == 0),
                                    stop=(q4 == 3 and c8 == 7),
                                )
                    for oc in range(8):
                        nc.vector.scalar_tensor_tensor(
                            r2[:, oc, lo:lo + HW_], mps[oc][:],
                            bpr_sb[:, oc:oc + 1], n_sb[:, oc, lo:lo + HW_],
                            op0=OP.add, op1=OP.add,
                        )
                layernorm(r2, r2, g2_sb, b2_sb, f"ln2{hh}", lo, HW_)
                out_v = out_d.rearrange("(c p) t -> p c t", p=P)
                for c8 in range(8):
                    nc.sync.dma_start(out_v[:, c8, lo:lo + HW_],
                                      r2[:, c8, lo:lo + HW_])
            wprp_cm.__exit__(None, None, None)
            wfcp_cm.__exit__(None, None, None)
            pN_cm.__exit__(None, None, None)
            pR_cm.__exit__(None, None, None)

    nc.compile()
    return nc


def _prep_shared(w_attn, b_attn, w_o, b_o, ln1_g, ln1_b, w_fc, b_fc, w_pr, b_pr,
                 ln2_g, ln2_b):
    w_attn = np.asarray(w_attn, np.float32)
    b_attn = np.asarray(b_attn, np.float32)
    w_o_f = np.asarray(w_o, np.float32)
    b_v = b_attn[2 * D:]
    b_o_eff = (np.asarray(b_o, np.float32) + b_v @ w_o_f).astype(np.float32)
    mask = np.where(
        np.arange(P)[:, None] > np.arange(P)[None, :], MASK_NEG, 0.0
    ).astype(BF16)  # [ki, qj]: mask keys above the diagonal
    shared = {
        "w_fc": np.asarray(w_fc, np.float32).astype(BF16),
        "w_pr": np.asarray(w_pr, np.float32).astype(BF16),
        "b_o": b_o_eff,
        "b_fc": np.asarray(b_fc, np.float32),
        "b_pr": np.asarray(b_pr, np.float32),
        "g1": np.asarray(ln1_g, np.float32),
        "b1": np.asarray(ln1_b, np.float32),
        "g2": np.asarray(ln2_g, np.float32),
        "b2": np.asarray(ln2_b, np.float32),
        "ident": np.eye(P, dtype=np.float32).astype(BF16),
        "maskm": mask,
    }
    return shared, w_attn, b_attn, w_o_f


def _own_token_idx(j):
    """RS-order tokens of core 4b+j: segment j of each 512-block."""
    return np.concatenate(
        [np.arange(TOK * g + P * j, TOK * g + P * (j + 1)) for g in range(NTB)]
    )


def kernel(x, w_attn, b_attn, w_o, b_o, ln1_g, ln1_b, w_fc, b_fc, w_pr, b_pr,
           ln2_g, ln2_b, _trace=False):
    from concourse.bass_utils import run_bass_kernel_spmd

    if "nc" not in _CACHE:
        _CACHE["nc"] = _build()
    nc = _CACHE["nc"]

    x = np.asarray(x, np.float32)
    shared, w_attn_f, b_attn_f, w_o_f = _prep_shared(
        w_attn, b_attn, w_o, b_o, ln1_g, ln1_b, w_fc, b_fc, w_pr, b_pr,
        ln2_g, ln2_b)

    xTb_bf = [np.ascontiguousarray(x[b].T).astype(BF16) for b in range(B)]

    in_maps = []
    idxs = []
    for c in range(N_CORES):
        b, j = c // 4, c % 4
        idx = _own_token_idx(j)
        idxs.append((b, idx))
        m = dict(shared)
        m["xTb"] = xTb_bf[b]
        m["xTf"] = np.ascontiguousarray(x[b, idx, :].T)
        m["w_q"] = np.ascontiguousarray(
            w_attn_f[:, 256 * j:256 * (j + 1)]).astype(BF16)
        m["w_k"] = np.ascontiguousarray(
            w_attn_f[:, D + 256 * j:D + 256 * (j + 1)]).astype(BF16)
        m["w_v"] = np.ascontiguousarray(
            w_attn_f[:, 2 * D + 256 * j:2 * D + 256 * (j + 1)]).astype(BF16)
        m["w_oo"] = np.ascontiguousarray(
            w_o_f[256 * j:256 * (j + 1), :]).astype(BF16)
        m["b_q"] = np.ascontiguousarray(b_attn_f[256 * j:256 * (j + 1)])
        m["b_k"] = np.ascontiguousarray(b_attn_f[D + 256 * j:D + 256 * (j + 1)])
        in_maps.append(m)

    res = run_bass_kernel_spmd(
        nc, in_maps, core_ids=list(range(N_CORES)), trace=_trace
    )
    if _trace:
        _CACHE["exec_time_ns"] = res.exec_time_ns
        _CACHE["insts_and_trace"] = res.instructions_and_trace

    out = np.empty((B, S, D), np.float32)
    for c in range(N_CORES):
        b, idx = idxs[c]
        out[b, idx, :] = res.results[c]["out"].T
    return out


# revision 24
# speedup vs baseline: 1.1999x; 1.0429x over previous
"""Dense transformer block (attention + post-LN MLP) on 8 trn2 NeuronCores.

Head-parallel sharding: core c = 4b+j handles heads {4j..4j+3} of batch
b over the full 2048-token sequence, so every core runs the same
uniform causal-attention program (no runtime Switch) and there is no
K/V exchange: each core computes q/k/v for its own heads over all
tokens. Each core also computes its heads' partial contribution to the
attention projection (o @ w_o rows) for ALL tokens; four pipelined
ReduceScatters (one per 512-token block, issued as each block's
attention completes) sum the partials across the 4-core batch group and
deliver each core its own 128-token segment — at a core-independent
address. The MLP is token-sharded over the RS-assigned tokens. Weights
replicated (bf16); activations feature-major (x^T) end-to-end. qkv /
attention / softmax-exp / w_o-partial work is interleaved in emission
order so the scalar-engine exp stream hides under tensor-engine
matmuls.
"""

import numpy as np
import ml_dtypes

BF16 = ml_dtypes.bfloat16

N_CORES = 8
B, S, D = 2, 2048, 1024
H, HD = 16, 64
F = 4 * D
TOK = 512            # tokens owned per core (MLP/output shard)
P = 128
NTB = 4              # 512-token blocks per sequence
VW = 66              # per-head V row width: 64 v + ones col + pad
MASK_NEG = -80000.0  # -> -79872 in bf16; /8 => exp underflows to exactly 0
LN_EPS = 1e-5

_CACHE = {}


def _patched_insert_act_table_loads(self):
    """Instance-level replacement for Bacc.insert_act_table_loads that
    removes Exp/Ln from every table set except natural_log_exp_and_others,
    so the softmax reciprocal's Ln/Exp alternation resolves to ONE set and
    the per-head ACT_TABLE_LOAD thrash (~1.3us each) disappears."""
    import bass_rust as _bass_rust
    import concourse.mybir as mybir
    from concourse.hw_specs import get_activation_tables

    has_activation = any(
        isinstance(i, mybir.InstActivation)
        for b in self.main_func.blocks
        for i in b.instructions
    )
    if not has_activation:
        return
    tabs = get_activation_tables(self.m.arch)
    AF = mybir.ActivationFunctionType
    if "natural_log_exp_and_others" in tabs:
        for name, fns in tabs.items():
            if name != "natural_log_exp_and_others":
                fns.discard(AF.Exp)
                fns.discard(AF.Ln)
    _bass_rust.insert_act_table_loads(self, list(tabs.items()))


def _build():
    import types

    import concourse.bass as bass
    import concourse.mybir as mybir
    import concourse.tile as tile
    from concourse import bacc

    dt = mybir.dt
    AF = mybir.ActivationFunctionType
    OP = mybir.AluOpType

    nc = bacc.Bacc(
        "TRN2",
        target_bir_lowering=False,
        debug=False,
        enable_asserts=True,
        num_devices=N_CORES,
    )
    try:
        nc.insert_act_table_loads = types.MethodType(
            _patched_insert_act_table_loads, nc
        )
    except Exception:
        pass

    def din(name, shape, dty):
        return nc.dram_tensor(name, shape, dty, kind="ExternalInput").ap()

    xTb = din("xTb", [D, S], dt.bfloat16)       # own batch, feature-major
    xTf = din("xTf", [D, TOK], dt.float32)      # own (RS-order) tokens
    w_q = din("w_q", [D, 256], dt.bfloat16)     # own 4 heads
    w_k = din("w_k", [D, 256], dt.bfloat16)
    w_v = din("w_v", [D, 256], dt.bfloat16)
    w_oo = din("w_oo", [256, D], dt.bfloat16)   # w_o rows of own heads
    w_fc = din("w_fc", [D, F], dt.bfloat16)
    w_pr = din("w_pr", [F, D], dt.bfloat16)
    b_q = din("b_q", [256], dt.float32)
    b_k = din("b_k", [256], dt.float32)
    b_o = din("b_o", [D], dt.float32)           # b_o_eff (v-bias folded)
    b_fc = din("b_fc", [F], dt.float32)
    b_pr = din("b_pr", [D], dt.float32)
    g1 = din("g1", [D], dt.float32)
    b1 = din("b1", [D], dt.float32)
    g2 = din("g2", [D], dt.float32)
    b2 = din("b2", [D], dt.float32)
    ident_d = din("ident", [P, P], dt.bfloat16)
    maskm_d = din("maskm", [P, P], dt.bfloat16)
    out_d = nc.dram_tensor("out", [D, TOK], dt.float32, kind="ExternalOutput").ap()

    GROUPS = [[0, 1, 2, 3], [4, 5, 6, 7]]

    with tile.TileContext(nc) as tc:
        from contextlib import ExitStack

        ctx = ExitStack()
        with ctx:
            c_pool = ctx.enter_context(tc.tile_pool(name="consts", bufs=1))
            dram = ctx.enter_context(tc.tile_pool(name="dram", bufs=1, space="DRAM"))

            # ---- constants ----
            ident = c_pool.tile([P, P], dt.bfloat16, tag="ident")
            nc.sync.dma_start(ident[:], ident_d[:])
            maskm = c_pool.tile([P, P], dt.bfloat16, tag="maskm")
            nc.sync.dma_start(maskm[:], maskm_d[:])
            ones128_bf = c_pool.tile([P, 1], dt.bfloat16, tag="ones128")
            nc.vector.memset(ones128_bf[:], 1.0)
            ones128_f = c_pool.tile([P, 1], dt.float32, tag="ones128f")
            nc.vector.memset(ones128_f[:], 1.0)
            ones1_f = c_pool.tile([1, P], dt.float32, tag="ones1f")
            nc.vector.memset(ones1_f[:], 1.0)
            ones_hi = c_pool.tile([65, 64], dt.bfloat16, tag="oneshi")
            nc.vector.memset(ones_hi[64:65, :], 1.0)

            bq_sb = c_pool.tile([P, 2], dt.float32, tag="bq")
            nc.sync.dma_start(bq_sb[:], b_q.rearrange("(c p) -> p c", p=P))
            bk_sb = c_pool.tile([P, 2], dt.float32, tag="bk")
            nc.sync.dma_start(bk_sb[:], b_k.rearrange("(c p) -> p c", p=P))
            bo_sb = c_pool.tile([P, 8], dt.float32, tag="bo")
            nc.sync.dma_start(bo_sb[:], b_o.rearrange("(c p) -> p c", p=P))
            bfc_sb = c_pool.tile([P, 32], dt.float32, tag="bfc")
            nc.sync.dma_start(bfc_sb[:], b_fc.rearrange("(c p) -> p c", p=P))
            bpr_sb = c_pool.tile([P, 8], dt.float32, tag="bpr")
            nc.sync.dma_start(bpr_sb[:], b_pr.rearrange("(c p) -> p c", p=P))
            g1_sb = c_pool.tile([P, 8], dt.float32, tag="g1")
            nc.sync.dma_start(g1_sb[:], g1.rearrange("(c p) -> p c", p=P))
            b1_sb = c_pool.tile([P, 8], dt.float32, tag="b1")
            nc.sync.dma_start(b1_sb[:], b1.rearrange("(c p) -> p c", p=P))
            g2_sb = c_pool.tile([P, 8], dt.float32, tag="g2")
            nc.sync.dma_start(g2_sb[:], g2.rearrange("(c p) -> p c", p=P))
            b2_sb = c_pool.tile([P, 8], dt.float32, tag="b2")
            nc.sync.dma_start(b2_sb[:], b2.rearrange("(c p) -> p c", p=P))

            # ---- ReduceScatter buffers: one per 512-token block ----
            rs_in = [dram.tile([4 * D * P], dt.bfloat16, tag=f"rsi{g}",
                               name=f"rsi{g}") for g in range(NTB)]
            rs_out = [dram.tile([D * P], dt.bfloat16, tag=f"rso{g}",
                                name=f"rso{g}") for g in range(NTB)]

            # tiny warm-up collective: absorbs the runtime's first-
            # collective global barrier while qkv is still starting up,
            # so the pipelined RS chain below isn't delayed behind it.
            wu_in = dram.tile([256], dt.bfloat16, tag="wui", name="wui")
            wu_out = dram.tile([4, 256], dt.bfloat16, tag="wuo", name="wuo")
            nc.gpsimd.collective_compute(
                "AllGather", mybir.AluOpType.bypass,
                replica_groups=GROUPS,
                ins=[wu_in.opt()], outs=[wu_out.opt()],
            )

            # ---- long-lived activations ----
            pR_cm = tc.tile_pool(name="pR", bufs=1)
            pR = pR_cm.__enter__()
            r1 = pR.tile([P, 8, TOK], dt.float32, tag="r1")
            xf = pR.tile([P, 8, TOK], dt.float32, tag="xf")
            rsg = pR.tile([P, 8, TOK], dt.bfloat16, tag="rsg")

            pA_cm = tc.tile_pool(name="pA", bufs=1)
            pA = pA_cm.__enter__()
            qT = pA.tile([P, 2, S], dt.bfloat16, tag="qT")
            kT = pA.tile([P, 2, S], dt.bfloat16, tag="kT")
            v_ones = pA.tile([P, 16, 4 * VW], dt.bfloat16, tag="v_ones")
            o_cat = [pA.tile([P, S], dt.bfloat16, tag=f"o_cat{i}", name=f"o_cat{i}")
                     for i in range(2)]
            woo_sb = pA.tile([P, 2, D], dt.bfloat16, tag="woo")

            v4 = v_ones.rearrange("p k (h w) -> p k h w", w=VW)
            nc.vector.memset(v4[:, :, :, 64:66], 0.0)
            nc.vector.memset(v4[:, :, :, 64:65], 1.0)

            # ---- qkv inputs ----
            xw_cm = tc.tile_pool(name="xw", bufs=1)
            xw = xw_cm.__enter__()
            wk_sb = xw.tile([P, 8, 256], dt.bfloat16, tag="wk")
            nc.sync.dma_start(wk_sb[:], w_k.rearrange("(c p) f -> p c f", p=P))
            wq_sb = xw.tile([P, 8, 256], dt.bfloat16, tag="wq")
            nc.sync.dma_start(wq_sb[:], w_q.rearrange("(c p) f -> p c f", p=P))
            wv_sb = xw.tile([P, 8, 256], dt.bfloat16, tag="wv")
            nc.sync.dma_start(wv_sb[:], w_v.rearrange("(c p) f -> p c f", p=P))
            nc.sync.dma_start(woo_sb[:], w_oo.rearrange("(c p) f -> p c f", p=P))
            xbp_cm = tc.tile_pool(name="xbp", bufs=3)
            xbp = xbp_cm.__enter__()
            xbv = xTb.rearrange("(c p) t -> p c t", p=P)

            # prefetch for the post-attention phases
            nc.sync.dma_start(xf[:], xTf.rearrange("(c p) t -> p c t", p=P))

            # ---- attention working pools ----
            # qkv/w_o psum shares the scores pool ("sps" tag): 2 slots x
            # 2 banks + oT 2 slots x 2 banks = all 8 banks.
            s_ps_cm = tc.tile_pool(name="att_s", bufs=2, space="PSUM")
            s_ps = s_ps_cm.__enter__()
            o_ps_cm = tc.tile_pool(name="att_o", bufs=2, space="PSUM")
            o_ps = o_ps_cm.__enter__()
            atmp_cm = tc.tile_pool(name="att_tmp", bufs=3)
            atmp = atmp_cm.__enter__()
            atmp2_cm = tc.tile_pool(name="att_tmp2", bufs=2)
            atmp2 = atmp2_cm.__enter__()
            wop_cm = tc.tile_pool(name="wop", bufs=2)
            wop = wop_cm.__enter__()

            def qkv_tb(tb):
                """q/k/v for token block tb (512 tokens), own 4 heads."""
                xbt = xbp.tile([P, 8, TOK], dt.bfloat16, tag="xb")
                nc.sync.dma_start(
                    xbt[:], xbv[:, :, tb * TOK:(tb + 1) * TOK])
                for hp in range(2):
                    ps = s_ps.tile([P, TOK], dt.float32, tag="sps")
                    for c8 in range(8):
                        nc.tensor.matmul(
                            ps[:],
                            lhsT=wk_sb[:, c8, hp * P:(hp + 1) * P],
                            rhs=xbt[:, c8, :],
                            start=(c8 == 0), stop=(c8 == 7),
                        )
                    nc.scalar.activation(
                        kT[:, hp, tb * TOK:(tb + 1) * TOK], ps[:],
                        AF.Identity, bias=bk_sb[:, hp:hp + 1],
                    )
                for kb in range(4):
                    kcg = tb * 4 + kb
                    ps = s_ps.tile([P, 256], dt.float32, tag="sps")
                    for c8 in range(8):
                        nc.tensor.matmul(
                            ps[:],
                            lhsT=xbt[:, c8, kb * P:(kb + 1) * P],
                            rhs=wv_sb[:, c8, :],
                            start=(c8 == 0), stop=(c8 == 7),
                        )
                    nc.vector.tensor_copy(
                        v4[:, kcg, :, 0:64],
                        ps[:].rearrange("p (h w) -> p h w", w=64),
                    )
                for hp in range(2):
                    ps = s_ps.tile([P, TOK], dt.float32, tag="sps")
                    for c8 in range(8):
                        nc.tensor.matmul(
                            ps[:],
                            lhsT=wq_sb[:, c8, hp * P:(hp + 1) * P],
                            rhs=xbt[:, c8, :],
                            start=(c8 == 0), stop=(c8 == 7),
                        )
                    nc.scalar.activation(
                        qT[:, hp, tb * TOK:(tb + 1) * TOK], ps[:],
                        AF.Identity, bias=bq_sb[:, hp:hp + 1],
                    )

            def attn_core(hp, g):
                """Causal attention for head-pair hp, query block g (512 q).
                AV matmuls are emitted one key-chunk behind the scores so
                the PE never stalls on the scalar-engine exp."""
                nkc = 4 * g + 4
                oT = o_ps.tile([65, 2, TOK], dt.float32, tag="oT")
                pend = None

                def emit_av(item):
                    aT_, kc_ = item
                    diag_ = kc_ >= 4 * g
                    qs_ = (kc_ - 4 * g) * P if diag_ else 0
                    for h2 in range(2):
                        h = 2 * hp + h2
                        nc.tensor.matmul(
                            oT[:, h2, qs_:],
                            lhsT=v_ones[:, kc_, h * VW:h * VW + 65],
                            rhs=aT_[:, h2, qs_:],
                            start=(kc_ == 0), stop=(kc_ == nkc - 1),
                        )

                for kc in range(nkc):
                    diag = kc >= 4 * g
                    qs = (kc - 4 * g) * P if diag else 0
                    sps = s_ps.tile([P, 2, TOK], dt.float32, tag="sps")
                    for h2 in range(2):
                        nc.tensor.matmul(
                            sps[:, h2, qs:],
                            lhsT=kT[64 * h2:64 * (h2 + 1), hp, kc * P:(kc + 1) * P],
                            rhs=qT[64 * h2:64 * (h2 + 1), hp,
                                   g * TOK + qs:(g + 1) * TOK],
                            start=True, stop=not diag,
                            tile_position=(64 * h2, 0),
                        )
                        if diag:
                            nc.tensor.matmul(
                                sps[:, h2, qs:qs + P],
                                lhsT=ident[:], rhs=maskm[:],
                                start=False, stop=True,
                            )
                    aT = atmp.tile([P, 2, TOK], dt.bfloat16, tag="aT")
                    nc.scalar.activation(
                        aT[:, :, qs:], sps[:, :, qs:], AF.Exp, scale=0.125
                    )
                    if pend is not None:
                        emit_av(pend)
                    pend = (aT, kc)
                emit_av(pend)
                return oT

            def attn_norm(hp, g, oT):
                # softmax denominators -> reciprocals via ln/exp, broadcast
                lnrow = atmp2.tile([65, 2, TOK], dt.float32, tag="lnrow")
                nc.scalar.activation(lnrow[64:65, :, :], oT[64:65, :, :], AF.Ln)
                rrow = atmp2.tile([65, 2, TOK], dt.bfloat16, tag="rrow")
                nc.scalar.activation(
                    rrow[64:65, :, :], lnrow[64:65, :, :], AF.Exp, scale=-1.0
                )
                rep = s_ps.tile([P, 2, TOK], dt.float32, tag="sps")
                for h2 in range(2):
                    nc.tensor.matmul(
                        rep[0:64, h2, :],
                        lhsT=ones_hi[64:65, :], rhs=rrow[64:65, h2, :],
                        start=True, stop=True,
                    )
                rep_sb = atmp2.tile([64, 2, TOK], dt.float32, tag="rep_sb")
                nc.scalar.activation(rep_sb[:], rep[0:64, :, :], AF.Copy)
                nc.vector.tensor_tensor(
                    o_cat[hp][0:64, g * TOK:(g + 1) * TOK],
                    oT[0:64, 0, :], rep_sb[:, 0, :], OP.mult,
                )
                ot = atmp2.tile([64, TOK], dt.bfloat16, tag="o_tmp")
                nc.vector.tensor_tensor(ot[:], oT[0:64, 1, :], rep_sb[:, 1, :],
                                        OP.mult)
                nc.sync.dma_start(o_cat[hp][64:128, g * TOK:(g + 1) * TOK], ot[:])

            def wo_partial(g):
                """Own-heads partial of o @ w_o for token block g + RS."""
                stage = wop.tile([P, 8, TOK], dt.bfloat16, tag="wostage")
                for op2 in range(4):
                    ps = s_ps.tile([P, 2, TOK], dt.float32, tag="sps")
                    for ocl in range(2):
                        oc = 2 * op2 + ocl
                        for ic in range(2):
                            nc.tensor.matmul(
                                ps[:, ocl, :],
                                lhsT=woo_sb[:, ic, oc * P:(oc + 1) * P],
                                rhs=o_cat[ic][:, g * TOK:(g + 1) * TOK],
                                start=(ic == 0), stop=(ic == 1),
                            )
                    nc.vector.tensor_copy(stage[:, 2 * op2:2 * op2 + 2, :], ps[:])
                rsv = rs_in[g].rearrange("(r c p t) -> r p c t", r=4, c=8, p=P)
                for r in range(4):
                    nc.sync.dma_start(
                        rsv[r], stage[:, :, r * P:(r + 1) * P])
                nc.gpsimd.collective_compute(
                    "ReduceScatter", mybir.AluOpType.add,
                    replica_groups=GROUPS,
                    ins=[rs_in[g].opt()], outs=[rs_out[g].opt()],
                )

            # ---- interleaved qkv + attention + projection schedule ----
            # norms of block g are deferred past qkv(g+1) so their ACT
            # chain overlaps qkv matmuls instead of stalling the PE.
            qkv_tb(0)
            o00 = attn_core(0, 0)
            o10 = attn_core(1, 0)
            prev = (o00, o10)
            for g in range(1, NTB):
                qkv_tb(g)
                attn_norm(0, g - 1, prev[0])
                attn_norm(1, g - 1, prev[1])
                wo_partial(g - 1)
                prev = (attn_core(0, g), attn_core(1, g))
            attn_norm(0, NTB - 1, prev[0])
            attn_norm(1, NTB - 1, prev[1])
            wo_partial(NTB - 1)

            # close attention pools in LIFO order
            wop_cm.__exit__(None, None, None)
            atmp2_cm.__exit__(None, None, None)
            atmp_cm.__exit__(None, None, None)
            o_ps_cm.__exit__(None, None, None)
            s_ps_cm.__exit__(None, None, None)
            xbp_cm.__exit__(None, None, None)
            xw_cm.__exit__(None, None, None)

            # gather RS outputs: own 128-token segment of each block g
            for g in range(NTB):
                nc.sync.dma_start(
                    rsg[:, :, g * P:(g + 1) * P],
                    rs_out[g].rearrange("(c p t) -> p c t", c=8, p=P),
                )

            # ============ layernorm (feature-major, partition reduce) ====
            # src: [P, 8, TOK] f32 tile; dst_ap(c8) -> AP for chunk c8's
            # [P, wd] output (any dtype). Column subrange [lo, lo+wd).
            # s1 uses an fp32 matmul directly on src (no bf16 staging);
            # squares computed on ACT to keep the DVE queue short.
            def layernorm(src, dst_ap, g_sb, b_sb, tag, lo, wd):
                with (
                    tc.tile_pool(name=f"ln_{tag}", bufs=2) as lnp,
                    tc.tile_pool(name=f"lnps_{tag}", bufs=2, space="PSUM") as lnps,
                    tc.tile_pool(name=f"lnrep_{tag}", bufs=1, space="PSUM") as lnrep,
                ):
                    s1 = lnps.tile([1, wd], dt.float32, tag=f"s1_{tag}")
                    s2 = lnps.tile([1, wd], dt.float32, tag=f"s2_{tag}")
                    for c8 in range(8):
                        sq = lnp.tile([P, wd], dt.bfloat16, tag=f"sq_{tag}")
                        nc.scalar.activation(sq[:], src[:, c8, lo:lo + wd],
                                             AF.Square)
                        nc.tensor.matmul(
                            s1[:], lhsT=ones128_f[:], rhs=src[:, c8, lo:lo + wd],
                            start=(c8 == 0), stop=(c8 == 7),
                        )
                        nc.tensor.matmul(
                            s2[:], lhsT=ones128_bf[:], rhs=sq[:],
                            start=(c8 == 0), stop=(c8 == 7),
                        )
                    mu = lnp.tile([1, wd], dt.float32, tag=f"mu_{tag}")
                    nc.vector.tensor_scalar(mu[:], s1[:], 1.0 / D, None, OP.mult)
                    m2 = lnp.tile([1, wd], dt.float32, tag=f"m2_{tag}")
                    nc.vector.tensor_scalar(m2[:], s2[:], 1.0 / D, LN_EPS, OP.mult, OP.add)
                    var = lnp.tile([1, wd], dt.float32, tag=f"var_{tag}")
                    nc.vector.tensor_tensor(var[:], mu[:], mu[:], OP.mult)
                    nc.vector.tensor_tensor(var[:], m2[:], var[:], OP.subtract)
                    lnv = lnp.tile([1, wd], dt.float32, tag=f"lnv_{tag}")
                    nc.scalar.activation(lnv[:], var[:], AF.Ln)
                    rsmu = lnp.tile([1, 2, wd], dt.float32, tag=f"rsmu_{tag}")
                    nc.scalar.activation(rsmu[:, 0, :], lnv[:], AF.Exp, scale=-0.5)
                    nc.vector.tensor_tensor(
                        rsmu[:, 1, :], mu[:], rsmu[:, 0, :], OP.mult
                    )
                    rep = lnrep.tile([P, 2, wd], dt.float32, tag=f"rep_{tag}")
                    for i in range(2):
                        nc.tensor.matmul(
                            rep[:, i, :], lhsT=ones1_f[:], rhs=rsmu[:, i, :],
                            start=True, stop=True,
                        )
                    rep_sb = lnp.tile([P, 2, wd], dt.float32, tag=f"repsb_{tag}")
                    nc.scalar.activation(rep_sb[:], rep[:], AF.Copy)
                    for c8 in range(8):
                        t = lnp.tile([P, wd], dt.float32, tag=f"t_{tag}")
                        nc.vector.tensor_tensor(
                            t[:], src[:, c8, lo:lo + wd], rep_sb[:, 0, :], OP.mult
                        )
                        nc.vector.tensor_tensor(t[:], t[:], rep_sb[:, 1, :], OP.subtract)
                        nc.vector.tensor_scalar(
                            dst_ap(c8), t[:], g_sb[:, c8:c8 + 1],
                            b_sb[:, c8:c8 + 1], OP.mult, OP.add,
                        )

            pA_cm.__exit__(None, None, None)
            pN_cm = tc.tile_pool(name="pN", bufs=1)
            pN = pN_cm.__enter__()
            n_bf = [pN.tile([P, TOK], dt.bfloat16, tag=f"n_bf{i}", name=f"n_bf{i}")
                    for i in range(8)]
            h1g = pN.tile([P, 32, TOK], dt.bfloat16, tag="h1g")
            r2 = r1  # reuse: each column range of r1 is dead before r2 writes it

            # MLP weight pools + prefetch
            wfcp_cm = tc.tile_pool(name="wfc", bufs=2)
            wfcp = wfcp_cm.__enter__()

            def load_wfc(oq):
                wq_ = wfcp.tile([P, 8, 1024], dt.bfloat16, tag="wfcq")
                nc.sync.dma_start(
                    wq_[:],
                    w_fc[:, oq * 1024:(oq + 1) * 1024].rearrange(
                        "(c p) f -> p c f", p=P),
                )
                return wq_

            wprp_cm = tc.tile_pool(name="wpr", bufs=2)
            wprp = wprp_cm.__enter__()

            def load_wpr(q4):
                wq_ = wprp.tile([P, 8, D], dt.bfloat16, tag="wprq")
                nc.sync.dma_start(
                    wq_[:],
                    w_pr[q4 * 1024:(q4 + 1) * 1024, :].rearrange(
                        "(c p) f -> p c f", p=P),
                )
                return wq_

            wfc_q0 = load_wfc(0)

            # ==== tail, pipelined in two 256-token column halves:
            # half 0 (RS blocks 0,1) starts while RS(2)/RS(3) still fly;
            # each LN chain hides under the other half's matmuls.
            HW_ = TOK // 2

            def residual_half(hh):
                lo = HW_ * hh
                for oc in range(8):
                    nc.vector.scalar_tensor_tensor(
                        r1[:, oc, lo:lo + HW_], rsg[:, oc, lo:lo + HW_],
                        bo_sb[:, oc:oc + 1], xf[:, oc, lo:lo + HW_],
                        op0=OP.add, op1=OP.add,
                    )

            def fc_half(hh):
                lo = HW_ * hh
                with tc.tile_pool(name=f"fc_ps{hh}", bufs=4, space="PSUM") as fc_ps:
                    for oq in range(4):
                        wq_ = wfc_q0 if (hh == 0 and oq == 0) else load_wfc(oq)
                        for oc8 in range(8):
                            oc = oq * 8 + oc8
                            ps = fc_ps.tile([P, HW_], dt.float32, tag="fcps")
                            for c8 in range(8):
                                nc.tensor.matmul(
                                    ps[:],
                                    lhsT=wq_[:, c8, oc8 * P:(oc8 + 1) * P],
                                    rhs=n_bf[c8][:, lo:lo + HW_],
                                    start=(c8 == 0), stop=(c8 == 7),
                                )
                            nc.scalar.activation(
                                h1g[:, oc, lo:lo + HW_], ps[:], AF.Gelu,
                                bias=bfc_sb[:, oc:oc + 1],
                            )

            def pr_half(hh):
                lo = HW_ * hh
                with tc.tile_pool(name=f"pr_ps{hh}", bufs=1, space="PSUM") as pr_ps:
                    mps = [pr_ps.tile([P, HW_], dt.float32, tag=f"mps{i}",
                                      name=f"mps{hh}_{i}") for i in range(8)]
                    for q4 in range(4):
                        wq_ = load_wpr(q4)
                        for oc in range(8):
                            for c8 in range(8):
                                nc.tensor.matmul(
                                    mps[oc][:],
                                    lhsT=wq_[:, c8, oc * P:(oc + 1) * P],
                                    rhs=h1g[:, q4 * 8 + c8, lo:lo + HW_],
                                    start=(q4 == 0 and c8 == 0),
                                    stop=(q4 == 3 and c8 == 7),
                                )
                    for oc in range(8):
                        nc.vector.scalar_tensor_tensor(
                            r2[:, oc, lo:lo + HW_], mps[oc][:],
                            bpr_sb[:, oc:oc + 1], n_bf[oc][:, lo:lo + HW_],
                            op0=OP.add, op1=OP.add,
                        )

            out_v = out_d.rearrange("(c p) t -> p c t", p=P)

            residual_half(0)
            layernorm(r1, lambda c8: n_bf[c8][:, 0:HW_], g1_sb, b1_sb,
                      "ln1a", 0, HW_)
            fc_half(0)
            residual_half(1)
            layernorm(r1, lambda c8: n_bf[c8][:, HW_:TOK], g1_sb, b1_sb,
                      "ln1b", HW_, HW_)
            pr_half(0)
            layernorm(r2, lambda c8: r1[:, c8, 0:HW_], g2_sb, b2_sb,
                      "ln2a", 0, HW_)
            fc_half(1)
            for c8 in range(8):
                nc.sync.dma_start(out_v[:, c8, 0:HW_], r1[:, c8, 0:HW_])
            pr_half(1)
            layernorm(r2, lambda c8: r1[:, c8, HW_:TOK], g2_sb, b2_sb,
                      "ln2b", HW_, HW_)
            for c8 in range(8):
                nc.sync.dma_start(out_v[:, c8, HW_:TOK], r1[:, c8, HW_:TOK])
            wprp_cm.__exit__(None, None, None)
            wfcp_cm.__exit__(None, None, None)
            pN_cm.__exit__(None, None, None)
            pR_cm.__exit__(None, None, None)

    nc.compile()
    return nc


def _prep_shared(w_attn, b_attn, w_o, b_o, ln1_g, ln1_b, w_fc, b_fc, w_pr, b_pr,
                 ln2_g, ln2_b):
    w_attn = np.asarray(w_attn, np.float32)
    b_attn = np.asarray(b_attn, np.float32)
    w_o_f = np.asarray(w_o, np.float32)
    b_v = b_attn[2 * D:]
    b_o_eff = (np.asarray(b_o, np.float32) + b_v @ w_o_f).astype(np.float32)
    mask = np.where(
        np.arange(P)[:, None] > np.arange(P)[None, :], MASK_NEG, 0.0
    ).astype(BF16)  # [ki, qj]: mask keys above the diagonal
    shared = {
        "w_fc": np.asarray(w_fc, np.float32).astype(BF16),
        "w_pr": np.asarray(w_pr, np.float32).astype(BF16),
        "b_o": b_o_eff,
        "b_fc": np.asarray(b_fc, np.float32),
        "b_pr": np.asarray(b_pr, np.float32),
        "g1": np.asarray(ln1_g, np.float32),
        "b1": np.asarray(ln1_b, np.float32),
        "g2": np.asarray(ln2_g, np.float32),
        "b2": np.asarray(ln2_b, np.float32),
        "ident": np.eye(P, dtype=np.float32).astype(BF16),
        "maskm": mask,
    }
    return shared, w_attn, b_attn, w_o_f


def _own_token_idx(j):
    """RS-order tokens of core 4b+j: segment j of each 512-block."""
    return np.concatenate(
        [np.arange(TOK * g + P * j, TOK * g + P * (j + 1)) for g in range(NTB)]
    )


def kernel(x, w_attn, b_attn, w_o, b_o, ln1_g, ln1_b, w_fc, b_fc, w_pr, b_pr,
           ln2_g, ln2_b, _trace=False):
    from concourse.bass_utils import run_bass_kernel_spmd

    if "nc" not in _CACHE:
        _CACHE["nc"] = _build()
    nc = _CACHE["nc"]

    x = np.asarray(x, np.float32)
    shared, w_attn_f, b_attn_f, w_o_f = _prep_shared(
        w_attn, b_attn, w_o, b_o, ln1_g, ln1_b, w_fc, b_fc, w_pr, b_pr,
        ln2_g, ln2_b)

    xTb_bf = [np.ascontiguousarray(x[b].T).astype(BF16) for b in range(B)]

    in_maps = []
    idxs = []
    for c in range(N_CORES):
        b, j = c // 4, c % 4
        idx = _own_token_idx(j)
        idxs.append((b, idx))
        m = dict(shared)
        m["xTb"] = xTb_bf[b]
        m["xTf"] = np.ascontiguousarray(x[b, idx, :].T)
        m["w_q"] = np.ascontiguousarray(
            w_attn_f[:, 256 * j:256 * (j + 1)]).astype(BF16)
        m["w_k"] = np.ascontiguousarray(
            w_attn_f[:, D + 256 * j:D + 256 * (j + 1)]).astype(BF16)
        m["w_v"] = np.ascontiguousarray(
            w_attn_f[:, 2 * D + 256 * j:2 * D + 256 * (j + 1)]).astype(BF16)
        m["w_oo"] = np.ascontiguousarray(
            w_o_f[256 * j:256 * (j + 1), :]).astype(BF16)
        m["b_q"] = np.ascontiguousarray(b_attn_f[256 * j:256 * (j + 1)])
        m["b_k"] = np.ascontiguousarray(b_attn_f[D + 256 * j:D + 256 * (j + 1)])
        in_maps.append(m)

    res = run_bass_kernel_spmd(
        nc, in_maps, core_ids=list(range(N_CORES)), trace=_trace
    )
    if _trace:
        _CACHE["exec_time_ns"] = res.exec_time_ns
        _CACHE["insts_and_trace"] = res.instructions_and_trace

    out = np.empty((B, S, D), np.float32)
    for c in range(N_CORES):
        b, idx = idxs[c]
        out[b, idx, :] = res.results[c]["out"].T
    return out
